# revision 66
# baseline (speedup 1.0000x reference)
"""CenterNet loss (GT assignment + focal/giou losses) on 8 Trainium2 cores.

Sharding: core c handles image b = c//2 and half h = c%2 of EVERY FPN level
(so all 8 cores run an identical SPMD tile schedule). Each core produces
partial sums (giou_sum, reg_cnt, pos_sum, neg_sum, npos); a DRAM AllReduce
combines them and every core computes the final 3-vector.
"""

import numpy as np

import concourse.bass as bass
import concourse.bacc as bacc
import concourse.tile as tile
from concourse import ap_utils, mybir
from concourse.bass_utils import run_bass_kernel_spmd


def _pool_on(eng, nc, out, in_, func):
    """Emit InstPool (innermost-dim reduction) on the given engine.

    Pads the input AP to 5-D (hardware requirement) via unsqueeze."""
    while len(in_.shape) < 5:
        in_ = in_.unsqueeze(1)
    return eng.add_instruction(mybir.InstPool(
        name=f"I-{nc.next_id()}", func=func,
        ins=[eng.lower_ap(in_, opt=False)], outs=[eng.lower_ap(out)]))

F32 = mybir.dt.float32
I32 = mybir.dt.int32
AF = mybir.ActivationFunctionType
OP = mybir.AluOpType
AX = mybir.AxisListType

# ---------------- problem constants (hardcoded from the nn.Module) ---------
B, NBOX = 4, 64
STRIDES = (8, 16, 32, 64, 128)
LEVEL_HW = ((128, 128), (64, 64), (32, 32), (16, 16), (8, 8))
SIZES = ((0.0, 80.0), (64.0, 160.0), (128.0, 320.0), (256.0, 640.0), (512.0, 1e7))
LOC = [h * w for h, w in LEVEL_HW]          # [16384, 4096, 1024, 256, 64]
M_IMG = sum(LOC)                            # 21824
M_TOT = B * M_IMG                           # 87296
BASE = [0, 65536, 81920, 86016, 87040]      # global level bases (level-major)
HALF = [m // 2 for m in LOC]                # per-core per-level loc counts
NT = 86                                     # 128-loc tiles per core
NV = sum(HALF)                              # 10912 valid locs per core
NPAD = NT * 128                             # 11008
INF = 1e8
MIN_RADIUS2 = 16.0
DELTA = (1 - 0.8) / (1 + 0.8)
K_R2 = float(np.float32(DELTA ** 2 * 2))    # radius2 = max(K_R2*area, 16)
SIG_LO = float(np.float32(1e-4))
SIG_HI = float(np.float32(1.0 - 1e-4))
EPS_AC = float(np.float32(1e-7))
IGNORE_HIGH_FP = 0.85
MAGIC = 8388608.0  # 2^23: u+MAGIC-MAGIC rounds u to nearest int (u < 2^22)
M15 = 12582912.0   # 1.5*2^23: (u+(M15-.5))-M15 = floor(u) for non-half-int u
# supergroups: (tile0, n_tiles, level); all tiles in a group share a level
SG = [(i * 16, 16, 0) for i in range(4)] + [
    (64, 16, 1), (80, 4, 2), (84, 1, 3), (85, 1, 4)]
# tiles per level: L0 t0-63, L1 64-79, L2 80-83, L3 84, L4 85 (32 valid rows)

N_CORES = 8


def _pack(vec):
    """[NPAD] (loc j = t*128+p) -> [128, NT] with [p, t] layout."""
    return np.ascontiguousarray(vec.reshape(NT, 128).T)


def _grids_per_level():
    gs = []
    for (h, w), s in zip(LEVEL_HW, STRIDES):
        ys, xs = np.meshgrid(np.arange(h) * s, np.arange(w) * s, indexing="ij")
        g = np.stack([xs.reshape(-1), ys.reshape(-1)], 1).astype(np.float32) + s // 2
        gs.append(g)
    return gs


def _half_concat(per_level_fn, h):
    """Concat per-level arrays for half h, pad to NPAD."""
    parts = [per_level_fn(l, h) for l in range(5)]
    cat = np.concatenate(parts, 0)
    pad_shape = (NPAD - NV,) + cat.shape[1:]
    return np.concatenate([cat, np.zeros(pad_shape, cat.dtype)], 0)


_GRIDS = _grids_per_level()


def _build_locstat(h):
    """[128, 8, NT]: planes gx, gy, gx, gy, -gx, -gy, valid, inv_s."""
    g = _half_concat(lambda l, hh: _GRIDS[l][hh * HALF[l]:(hh + 1) * HALF[l]], h)
    gx, gy = g[:, 0], g[:, 1]
    valid = np.zeros(NPAD, np.float32)
    valid[:NV] = 1.0
    inv_s = _half_concat(
        lambda l, hh: np.full(HALF[l], 1.0 / STRIDES[l], np.float32), h)
    inv_s[NV:] = 1.0
    planes = [gx, gy, gx, gy, -gx, -gy, valid, inv_s]
    out = np.stack([_pack(p.astype(np.float32)) for p in planes], 1)
    return np.ascontiguousarray(out)  # [128, 8, NT]


_LOCSTAT = [_build_locstat(0), _build_locstat(1)]


def _shard_idx(b, h):
    """Global level-major indices of core (b, h)'s NV locations."""
    parts = [BASE[l] + b * LOC[l] + h * HALF[l] + np.arange(HALF[l])
             for l in range(5)]
    return np.concatenate(parts, 0)


_SHARD_IDX = {(b, h): _shard_idx(b, h) for b in range(B) for h in range(2)}

# per-level constants [128, 8, 5]:
# inv_s, s, s/2, lo, hi, W, 4*lo^2, 4*hi^2 (squared-domain cared tests)
_LVLC = np.ascontiguousarray(np.broadcast_to(np.stack([
    np.array([1.0 / s for s in STRIDES], np.float32),
    np.array(STRIDES, np.float32),
    np.array([s / 2.0 for s in STRIDES], np.float32),
    np.array([r[0] for r in SIZES], np.float32),
    np.array([r[1] for r in SIZES], np.float32),
    np.array([w for (_, w) in LEVEL_HW], np.float32),
    np.array([4.0 * r[0] * r[0] for r in SIZES], np.float32),
    np.array([4.0 * r[1] * r[1] for r in SIZES], np.float32),
], 0), (128, 8, 5)).astype(np.float32))


# ------------------------------ device program -----------------------------

def build_nc(with_cc=False, dbg=False):
    nc = bacc.Bacc(trn_type="TRN2", num_devices=N_CORES)
    locst = nc.dram_tensor("locst", [128, 8, NT], F32, kind="ExternalInput")
    dyn = nc.dram_tensor("dyn", [128, 5, NT], F32, kind="ExternalInput")
    boxesT = nc.dram_tensor("boxesT", [4, NBOX], F32, kind="ExternalInput")
    boxesP = nc.dram_tensor("boxesP", [2 * NBOX, 4], F32, kind="ExternalInput")
    agnfull = nc.dram_tensor("agnfull", [M_TOT, 1], F32, kind="ExternalInput")
    corec = nc.dram_tensor("corec", [NBOX, 8], F32, kind="ExternalInput")
    lvlc = nc.dram_tensor("lvlc", [128, 8, 5], F32, kind="ExternalInput")
    out = nc.dram_tensor("out", [1, 8], F32, kind="ExternalOutput")
    if dbg:
        minddbg = nc.dram_tensor("minddbg", [128, NT], F32, kind="ExternalOutput")
        minwdbg = nc.dram_tensor("minwdbg", [128, NT], F32, kind="ExternalOutput")
        xtdbg = nc.dram_tensor("xtdbg", [128, 4, NT], F32, kind="ExternalOutput")
        posdbg = nc.dram_tensor("posdbg", [NBOX, 5], F32, kind="ExternalOutput")
        gvdbg = nc.dram_tensor("gvdbg", [NBOX, 5], F32, kind="ExternalOutput")
    vec, act, gps, sync = nc.vector, nc.scalar, nc.gpsimd, nc.sync

    with tile.TileContext(nc) as tc:
        with tc.tile_pool(name="const", bufs=1) as cp, \
             tc.tile_pool(name="work", bufs=3) as wp, \
             tc.tile_pool(name="ppool", bufs=3, space="PSUM") as pp:

            # ---------------- loads ----------------
            SL = cp.tile([128, 8, NT], F32)
            sync.dma_start(out=SL[:], in_=locst[:])
            DY = cp.tile([128, 5, NT], F32)
            sync.dma_start(out=DY[:], in_=dyn[:])
            BBh = boxesT[:]
            BB = cp.tile([128, 4, NBOX], F32)
            bt_bc = bass.AP(tensor=BBh.tensor, offset=BBh.offset,
                            ap=[[0, 128], [NBOX, 4], [1, NBOX]])
            sync.dma_start(out=BB[:], in_=bt_bc)
            BP = cp.tile([2 * NBOX, 4], F32)
            sync.dma_start(out=BP[:], in_=boxesP[:])
            CO = cp.tile([NBOX, 8], F32)
            sync.dma_start(out=CO[:], in_=corec[:])
            LV = cp.tile([128, 8, 5], F32)
            sync.dma_start(out=LV[:], in_=lvlc[:])
            # ---------------- per-box precompute ([128, 64] broadcast) -----
            from concourse.masks import make_identity
            IDT = cp.tile([128, 128], mybir.dt.bfloat16)
            make_identity(nc, IDT[:])
            x0, y0 = BB[:, 0, :], BB[:, 1, :]
            x1, y1 = BB[:, 2, :], BB[:, 3, :]
            CXY = cp.tile([128, 2, NBOX], F32)
            CX, CY = CXY[:, 0, :], CXY[:, 1, :]
            vec.tensor_tensor(out=CXY[:], in0=BB[:, 0:2, :], in1=BB[:, 2:4, :],
                              op=OP.add)
            vec.tensor_scalar(out=CXY[:], in0=CXY[:], scalar1=0.5, scalar2=None,
                              op0=OP.mult)
            # W2C = (w/2, h/2) per box; W2SQ = squared (bf16)
            W2C = cp.tile([128, 2, NBOX], F32)
            vec.tensor_tensor(out=W2C[:], in0=BB[:, 2:4, :], in1=BB[:, 0:2, :],
                              op=OP.subtract)
            vec.tensor_scalar(out=W2C[:], in0=W2C[:], scalar1=0.5, scalar2=None,
                              op0=OP.mult)
            W2SQ = cp.tile([128, 2, NBOX], mybir.dt.bfloat16)
            act.square(out=W2SQ[:], in_=W2C[:])
            S1 = cp.tile([128, NBOX], F32)
            S2 = cp.tile([128, NBOX], F32)
            # radius2 = max(K_R2 * area, 16);  IR2N = -1/radius2
            R2 = cp.tile([128, NBOX], F32)
            vec.tensor_tensor(out=S1[:], in0=x1, in1=x0, op=OP.subtract)  # w
            vec.tensor_tensor(out=S2[:], in0=y1, in1=y0, op=OP.subtract)  # h
            vec.tensor_tensor(out=R2[:], in0=S1[:], in1=S2[:], op=OP.mult)
            vec.tensor_scalar(out=R2[:], in0=R2[:], scalar1=K_R2,
                              scalar2=MIN_RADIUS2, op0=OP.mult, op1=OP.max)
            IR2N = cp.tile([128, NBOX], F32)
            vec.reciprocal(out=IR2N[:], in_=R2[:])
            vec.tensor_scalar(out=IR2N[:], in0=IR2N[:], scalar1=-1.0,
                              scalar2=None, op0=OP.mult)
            IR2NB = cp.tile([128, NBOX], mybir.dt.bfloat16)
            vec.tensor_copy(out=IR2NB[:], in_=IR2N[:])
            # S1 = w^2 + h^2 = (2*crit)^2 (cared tests done in squared domain)
            vec.tensor_tensor(out=S1[:], in0=S1[:], in1=S1[:], op=OP.mult)
            vec.tensor_tensor(out=S2[:], in0=S2[:], in1=S2[:], op=OP.mult)
            vec.tensor_tensor(out=S1[:], in0=S1[:], in1=S2[:], op=OP.add)
            # PBS [128, 4] = (x0, y0, -x1, -y1) boxes-on-partitions
            PBS = cp.tile([2 * NBOX, 4], F32)
            vec.tensor_copy(out=PBS[:, 0:2], in_=BP[:, 0:2])
            vec.tensor_scalar(out=PBS[:, 2:4], in0=BP[:, 2:4], scalar1=-1.0,
                              scalar2=None, op0=OP.mult)
            # bf16 hi/lo split of PBS (hi+lo covers f32 to ~2^-17 rel).
            # 5th column: (1, 0) so the one-hot matmul also yields the
            # match count (ties are averaged via the count column).
            BF16 = mybir.dt.bfloat16
            NSPL = 2
            PBS3 = cp.tile([NBOX, NSPL, 5], BF16)
            PR1 = cp.tile([NBOX, 4], F32)
            vec.memset(PBS3[:], 0.0)
            vec.memset(PBS3[:, 0, 4:5], 1.0)
            vec.tensor_copy(out=PBS3[:, 0, 0:4], in_=PBS[0:NBOX, :])
            vec.tensor_copy(out=PR1[:], in_=PBS3[:, 0, 0:4])  # hi back to f32
            vec.tensor_tensor(out=PR1[:], in0=PBS[0:NBOX, :], in1=PR1[:],
                              op=OP.subtract)
            vec.tensor_copy(out=PBS3[:, 1, 0:4], in_=PR1[:])
            # block-diagonal variant for paired-tile extraction:
            # rows 0:64 -> cols 0:5, rows 64:128 -> cols 5:10
            PBSD = cp.tile([128, NSPL, 10], BF16)
            vec.memset(PBSD[:], 0.0)
            vec.memset(PBSD[0:NBOX, 0, 4:5], 1.0)
            vec.memset(PBSD[NBOX:2 * NBOX, 0, 9:10], 1.0)
            SPL = cp.tile([128, NSPL, 4], BF16)
            PRF = cp.tile([128, 4], F32)
            vec.tensor_copy(out=SPL[:, 0, :], in_=PBS[:])
            vec.tensor_copy(out=PRF[:], in_=SPL[:, 0, :])
            vec.tensor_tensor(out=PRF[:], in0=PBS[:], in1=PRF[:],
                              op=OP.subtract)
            vec.tensor_copy(out=SPL[:, 1, :], in_=PRF[:])
            vec.tensor_copy(out=PBSD[0:NBOX, :, 0:4], in_=SPL[0:NBOX, :, :])
            vec.tensor_copy(out=PBSD[NBOX:2 * NBOX, :, 5:9],
                            in_=SPL[NBOX:2 * NBOX, :, :])
            # bf16 grid copy (lattice-exact: multiples of 4 <= 1020)
            BF16 = mybir.dt.bfloat16
            SLB = cp.tile([128, 2, NT], BF16)
            vec.tensor_copy(out=SLB[:], in_=SL[:, 0:2, :])
            # batched over levels: BCQ[l] = cared ? -1 : >=1 (bf16, max-combine);
            # CDALL = (cdisx, cdisy) per level, bf16 lattice-exact
            BCQ = []
            SF1 = cp.tile([128, NBOX], F32, tag="sf1", name="sf1")
            for l in range(5):
                lo, hi = SIZES[l]
                lo2, hi2 = 4.0 * lo * lo, 4.0 * hi * hi
                cn = cp.tile([128, NBOX], BF16, tag=f"bcq{l}", name=f"bcq{l}")
                vec.tensor_scalar(out=SF1[:], in0=S1[:], scalar1=lo2,
                                  scalar2=None, op0=OP.is_lt)
                vec.scalar_tensor_tensor(out=SF1[:], in0=S1[:],
                                         scalar=hi2, in1=SF1[:],
                                         op0=OP.is_gt, op1=OP.add)
                vec.tensor_scalar(out=cn[:], in0=SF1[:], scalar1=2.0,
                                  scalar2=-1.0, op0=OP.mult, op1=OP.add)
                BCQ.append(cn)

            def lvb(pl, shape):  # LV plane [128,5] -> bcast (128, d1, 5, 64)
                return (LV[:, pl, :].unsqueeze(1).broadcast_to((128, shape[1], 5))
                        .unsqueeze(3).broadcast_to(shape))

            B254 = (128, 2, 5, 64)
            UU = cp.tile([128, 2, 5, 64], F32, tag="uu", name="uu")
            R0 = cp.tile([128, 2, 5, 64], F32, tag="r0", name="r0")
            CC = cp.tile([128, 2, 5, 64], F32, tag="ccf", name="ccf")
            vec.tensor_tensor(
                out=UU[:], in0=CXY[:].unsqueeze(2).broadcast_to(B254),
                in1=lvb(0, B254), op=OP.mult)  # u = c/s
            vec.tensor_scalar(out=R0[:], in0=UU[:], scalar1=MAGIC,
                              scalar2=MAGIC, op0=OP.add, op1=OP.subtract)
            vec.tensor_tensor(out=CC[:], in0=R0[:], in1=UU[:], op=OP.is_gt)
            vec.tensor_tensor(out=R0[:], in0=R0[:], in1=CC[:], op=OP.subtract)
            CDALL = cp.tile([128, 2, 5, 64], BF16, tag="cdall", name="cdall")
            vec.tensor_tensor(out=R0[:], in0=R0[:], in1=lvb(1, B254),
                              op=OP.mult)
            vec.tensor_tensor(out=CDALL[:], in0=R0[:], in1=lvb(2, B254),
                              op=OP.add)  # floor(c/s)*s + s/2

            # ---------------- pos part (boxes on partitions, [64, *]) -------
            # runs on gpsimd: independent of the main pair loop
            cx = cp.tile([NBOX, 1], F32, tag="pcx", name="pcx")
            cy = cp.tile([NBOX, 1], F32, tag="pcy", name="pcy")
            vec.tensor_tensor(out=cx[:], in0=BP[0:NBOX, 0:1], in1=BP[0:NBOX, 2:3], op=OP.add)
            vec.tensor_scalar(out=cx[:], in0=cx[:], scalar1=0.5, scalar2=None,
                              op0=OP.mult)
            vec.tensor_tensor(out=cy[:], in0=BP[0:NBOX, 1:2], in1=BP[0:NBOX, 3:4], op=OP.add)
            vec.tensor_scalar(out=cy[:], in0=cy[:], scalar1=0.5, scalar2=None,
                              op0=OP.mult)
            pw = cp.tile([NBOX, 1], F32, tag="ppw", name="ppw")
            ph = cp.tile([NBOX, 1], F32, tag="pph", name="pph")
            vec.tensor_tensor(out=pw[:], in0=BP[0:NBOX, 2:3], in1=BP[0:NBOX, 0:1],
                              op=OP.subtract)
            vec.tensor_tensor(out=ph[:], in0=BP[0:NBOX, 3:4], in1=BP[0:NBOX, 1:2],
                              op=OP.subtract)
            vec.tensor_tensor(out=pw[:], in0=pw[:], in1=pw[:], op=OP.mult)
            vec.tensor_tensor(out=ph[:], in0=ph[:], in1=ph[:], op=OP.mult)
            vec.tensor_tensor(out=pw[:], in0=pw[:], in1=ph[:], op=OP.add)

            POSF = cp.tile([NBOX, 5], F32, tag="posf", name="posf")
            PM = cp.tile([NBOX, 5], F32, tag="pm", name="pm")
            RX = cp.tile([NBOX, 5], F32, tag="rx", name="rx")
            RY = cp.tile([NBOX, 5], F32, tag="ry", name="ry")
            # ci = floor(c/s) per level (round-then-correct)
            PU = cp.tile([NBOX, 5], F32, tag="pu", name="pu")
            PC2 = cp.tile([NBOX, 5], F32, tag="pc2", name="pc2")
            vec.tensor_tensor(out=PU[:], in0=cx[:].broadcast_to((NBOX, 5)),
                              in1=LV[0:NBOX, 0, :], op=OP.mult)
            vec.tensor_scalar(out=RX[:], in0=PU[:], scalar1=MAGIC,
                              scalar2=MAGIC, op0=OP.add, op1=OP.subtract)
            vec.tensor_tensor(out=PC2[:], in0=RX[:], in1=PU[:], op=OP.is_gt)
            vec.tensor_tensor(out=RX[:], in0=RX[:], in1=PC2[:], op=OP.subtract)
            vec.tensor_tensor(out=PU[:], in0=cy[:].broadcast_to((NBOX, 5)),
                              in1=LV[0:NBOX, 0, :], op=OP.mult)
            vec.tensor_scalar(out=RY[:], in0=PU[:], scalar1=MAGIC,
                              scalar2=MAGIC, op0=OP.add, op1=OP.subtract)
            vec.tensor_tensor(out=PC2[:], in0=RY[:], in1=PU[:], op=OP.is_gt)
            vec.tensor_tensor(out=RY[:], in0=RY[:], in1=PC2[:], op=OP.subtract)
            # pos = base + ci_y*W + ci_x, clamped
            vec.tensor_tensor(out=RY[:], in0=RY[:], in1=LV[0:NBOX, 5, :],
                              op=OP.mult)
            vec.tensor_tensor(out=RY[:], in0=RY[:], in1=RX[:], op=OP.add)
            vec.tensor_tensor(out=RY[:], in0=RY[:], in1=CO[:, 0:5], op=OP.add)
            vec.tensor_scalar(out=POSF[:], in0=RY[:], scalar1=0.0,
                              scalar2=float(M_TOT - 1), op0=OP.max, op1=OP.min)
            # PM = (crit >= lo) & (crit <= hi) per level, in squared domain:
            # pw = w^2+h^2 = (2*crit)^2 vs lvlc planes 6/7 = 4*lo^2 / 4*hi^2
            vec.tensor_tensor(out=RX[:], in0=pw[:].broadcast_to((NBOX, 5)),
                              in1=LV[0:NBOX, 6, :], op=OP.is_ge)
            vec.tensor_tensor(out=PM[:], in0=pw[:].broadcast_to((NBOX, 5)),
                              in1=LV[0:NBOX, 7, :], op=OP.is_le)
            vec.tensor_tensor(out=PM[:], in0=PM[:], in1=RX[:], op=OP.mult)
            POSI = cp.tile([NBOX, 5], I32, tag="posi", name="posi")
            vec.tensor_copy(out=POSI[:], in_=POSF[:])
            GV = cp.tile([NBOX, 5], F32, tag="gv", name="gv")
            for l in range(5):
                gps.indirect_dma_start(
                    out=GV[:, l:l + 1], out_offset=None, in_=agnfull[:],
                    in_offset=bass.IndirectOffsetOnAxis(ap=POSI[:, l:l + 1],
                                                        axis=0))


            # ---------------- main pair loop --------------------------------
            # negated-min convention: plane 0 = -min(wdist2), 1 = -min(d)
            MINWD = cp.tile([128, 2, NT], BF16)
            # selected (x0, y0, -x1, -y1, count) sums over argmin one-hot
            XT5 = cp.tile([128, 5, NT], F32)

            for (t0, G, l) in SG:
                s = float(STRIDES[l])
                s2 = float(s * s)
                sl = slice(t0, t0 + G)

                def bb1(t2d):  # [128,64] const -> [128,G,64]
                    return t2d.unsqueeze(1).broadcast_to((128, G, 64))

                # DFC = (gx-cx, gy-cy) fp32
                DFC = wp.tile([128, 2, G, 64], F32, tag="dfc", name="dfc")
                vec.tensor_tensor(
                    out=DFC[:],
                    in0=SL[:, 0:2, sl].unsqueeze(3).broadcast_to((128, 2, G, 64)),
                    in1=CXY[:].unsqueeze(2).broadcast_to((128, 2, G, 64)),
                    op=OP.subtract)
                # DFD = (gx-cdisx, gy-cdisy) bf16, lattice-exact
                DFD = wp.tile([128, 2, G, 64], BF16, tag="dfd", name="dfd")
                vec.tensor_tensor(
                    out=DFD[:],
                    in0=SLB[:, :, sl].unsqueeze(3).broadcast_to((128, 2, G, 64)),
                    in1=CDALL[:, :, l, :].unsqueeze(2).broadcast_to(
                        (128, 2, G, 64)),
                    op=OP.subtract)
                SQ2 = wp.tile([128, 2, G, 64], BF16, tag="sq2", name="sq2")
                act.square(out=SQ2[:], in_=DFC[:])
                SQD = wp.tile([128, 2, G, 64], BF16, tag="sqd", name="sqd")
                act.square(out=SQD[:], in_=DFD[:])
                # in-box test in squared domain: dfx^2 >= (w/2)^2 -> outside
                MXQ = wp.tile([128, 2, G, 64], BF16, tag="mxq", name="mxq")
                vec.tensor_tensor(
                    out=MXQ[:], in0=SQ2[:],
                    in1=W2SQ[:].unsqueeze(2).broadcast_to((128, 2, G, 64)),
                    op=OP.subtract)
                M4 = wp.tile([128, G, 64], BF16, tag="m4", name="m4")
                vec.tensor_tensor(out=M4[:], in0=MXQ[:, 0], in1=MXQ[:, 1],
                                  op=OP.max)
                # peak / 3x3 tests on squared bf16 lattice values
                MQ = wp.tile([128, G, 64], BF16, tag="mq", name="mq")
                vec.tensor_tensor(out=MQ[:], in0=SQD[:, 0], in1=SQD[:, 1],
                                  op=OP.max)
                # invalid <=> max(MQ - 1.5*s^2, M4, BCQ) >= 0 (3x3 inclusive:
                # MQ lattice jumps s^2 -> 4s^2, so -1.5s^2 keeps MQ==s^2 valid)
                V = wp.tile([128, G, 64], BF16, tag="vv", name="vv")
                vec.scalar_tensor_tensor(out=V[:], in0=MQ[:],
                                         scalar=-1.5 * s2, in1=M4[:],
                                         op0=OP.add, op1=OP.max)
                vec.tensor_tensor(out=V[:], in0=V[:], in1=bb1(BCQ[l][:]),
                                  op=OP.max)
                PEN = wp.tile([128, G, 64], BF16, tag="pen", name="pen")
                vec.tensor_scalar(out=PEN[:], in0=V[:], scalar1=0.0,
                                  scalar2=-INF, op0=OP.is_ge, op1=OP.mult)
                D2 = wp.tile([128, G, 64], BF16, tag="d2", name="d2")
                vec.tensor_tensor(out=D2[:], in0=SQ2[:, 0], in1=SQ2[:, 1],
                                  op=OP.add)
                DZ = wp.tile([128, G, 64], BF16, tag="dz", name="dz")
                vec.scalar_tensor_tensor(out=DZ[:], in0=MQ[:], scalar=0.0,
                                         in1=D2[:], op0=OP.not_equal,
                                         op1=OP.mult)
                # WDN/DN share one tile -> single paired reduce
                WDD = wp.tile([128, 2, G, 64], BF16, tag="wdd", name="wdd")
                vec.tensor_tensor(out=WDD[:, 0], in0=DZ[:], in1=bb1(IR2NB[:]),
                                  op=OP.mult)  # -wdist2
                vec.tensor_tensor(out=WDD[:, 1], in0=WDD[:, 0], in1=PEN[:],
                                  op=OP.add)
                vec.tensor_reduce(out=MINWD[:, :, sl], in_=WDD[:], axis=AX.X,
                                  op=OP.max)
                # direct one-hot (ties summed; normalized later via count col)
                OH = wp.tile([128, G, 64], mybir.dt.bfloat16, tag="oh", name="oh")
                vec.tensor_tensor(out=OH[:], in0=WDD[:, 1],
                                  in1=MINWD[:, 1, sl].unsqueeze(2).broadcast_to(
                                      (128, G, 64)), op=OP.is_equal)
                # rt extraction on PE: paired-tile transpose + block-diag rhs
                if G % 2 == 0:
                    P2 = G // 2
                    OHT = pp.tile([128, P2, 128], mybir.dt.bfloat16, tag="oht",
                                  name="oht")
                    for gg in range(P2):
                        nc.tensor.transpose(
                            OHT[:, gg, :],
                            OH[:, 2 * gg:2 * gg + 2, :].rearrange("p a b -> p (a b)"), IDT[:])
                    OHTS = wp.tile([128, P2, 128], mybir.dt.bfloat16,
                                   tag="ohts", name="ohts")
                    act.copy(out=OHTS[:], in_=OHT[:])
                    RTP = pp.tile([128, G, 5], F32, tag="rtp", name="rtp")
                    for gg in range(P2):
                        for k in range(NSPL):
                            nc.tensor.matmul(
                                out=RTP[:, 2 * gg:2 * gg + 2, :].rearrange("p a b -> p (a b)"),
                                lhsT=OHTS[:, gg, :], rhs=PBSD[:, k, :],
                                start=(k == 0), stop=(k == NSPL - 1))
                else:
                    OHT = pp.tile([64, G, 128], mybir.dt.bfloat16, tag="oht1",
                                  name="oht1", bufs=1)
                    for g in range(G):
                        nc.tensor.transpose(OHT[:, g, :], OH[:, g, :], IDT[:])
                    OHTS = wp.tile([64, G, 128], mybir.dt.bfloat16,
                                   tag="ohts1", name="ohts1")
                    act.copy(out=OHTS[:], in_=OHT[:])
                    RTP = pp.tile([128, G, 5], F32, tag="rtp", name="rtp")
                    for g in range(G):
                        for k in range(NSPL):
                            nc.tensor.matmul(out=RTP[:, g, :],
                                             lhsT=OHTS[:, g, :],
                                             rhs=PBS3[:, k, :],
                                             start=(k == 0),
                                             stop=(k == NSPL - 1))
                act.copy(out=XT5[:, :, sl], in_=RTP[:].transpose([0, 2, 1]))

            # pos-part tail: depends on the GV gathers, runs after the loop
            # so the in-order vector queue is not blocked mid-loop
            PPRED = cp.tile([NBOX, 5], F32, tag="ppred", name="ppred")
            act.activation(out=PPRED[:], in_=GV[:], func=AF.Sigmoid)
            vec.tensor_scalar(out=PPRED[:], in0=PPRED[:], scalar1=SIG_LO,
                              scalar2=SIG_HI, op0=OP.max, op1=OP.min)
            QQ = cp.tile([NBOX, 5], F32, tag="qq", name="qq")
            vec.tensor_scalar(out=QQ[:], in0=PPRED[:], scalar1=-1.0, scalar2=1.0,
                              op0=OP.mult, op1=OP.add)
            vec.tensor_tensor(out=QQ[:], in0=QQ[:], in1=QQ[:], op=OP.mult)
            LGP = cp.tile([NBOX, 5], F32, tag="lgp", name="lgp")
            act.activation(out=LGP[:], in_=PPRED[:], func=AF.Ln)
            vec.tensor_tensor(out=LGP[:], in0=LGP[:], in1=QQ[:], op=OP.mult)
            vec.tensor_tensor(out=LGP[:], in0=LGP[:], in1=PM[:], op=OP.mult)
            # gate odd cores to zero (pos part owned by even core of each image)
            vec.tensor_scalar(out=LGP[:], in0=LGP[:], scalar1=CO[:, 5:6],
                              scalar2=None, op0=OP.mult)
            vec.tensor_scalar(out=PM[:], in0=PM[:], scalar1=CO[:, 5:6],
                              scalar2=None, op0=OP.mult)
            POSS = cp.tile([NBOX, 1], F32, tag="poss", name="poss")
            vec.tensor_reduce(out=POSS[:], in_=LGP[:], axis=AX.X, op=OP.add)
            NPOS = cp.tile([NBOX, 1], F32, tag="npos", name="npos")
            vec.tensor_reduce(out=NPOS[:], in_=PM[:], axis=AX.X, op=OP.add)

            # ---------------- epilogue: per-location [128, NT] --------------
            AGN = DY[:, 0, :]
            VAL = SL[:, 6, :]
            ISV = SL[:, 7, :]

            def lt(tag):
                return wp.tile([128, NT], F32, tag=tag, name=tag)

            HM = lt("hm")
            act.activation(out=HM[:], in_=MINWD[:, 0, :], func=AF.Exp, scale=1.0)
            vec.scalar_tensor_tensor(out=HM[:], in0=HM[:], scalar=SIG_LO,
                                     in1=HM[:], op0=OP.is_ge, op1=OP.mult)
            NW = lt("nw")
            vec.tensor_scalar(out=NW[:], in0=HM[:], scalar1=-1.0, scalar2=1.0,
                              op0=OP.mult, op1=OP.add)
            vec.tensor_tensor(out=NW[:], in0=NW[:], in1=NW[:], op=OP.mult)
            vec.tensor_tensor(out=NW[:], in0=NW[:], in1=NW[:], op=OP.mult)
            PC = lt("pc")
            act.activation(out=PC[:], in_=AGN, func=AF.Sigmoid)
            vec.tensor_scalar(out=PC[:], in0=PC[:], scalar1=SIG_LO,
                              scalar2=SIG_HI, op0=OP.max, op1=OP.min)
            Q = lt("q")
            vec.tensor_scalar(out=Q[:], in0=PC[:], scalar1=-1.0, scalar2=1.0,
                              op0=OP.mult, op1=OP.add)
            act.activation(out=Q[:], in_=Q[:], func=AF.Ln)  # log(1-pred)
            P2 = lt("p2")
            vec.tensor_tensor(out=P2[:], in0=PC[:], in1=PC[:], op=OP.mult)
            T1 = lt("t1")
            vec.tensor_tensor(out=T1[:], in0=Q[:], in1=P2[:], op=OP.mult)
            vec.tensor_tensor(out=T1[:], in0=T1[:], in1=NW[:], op=OP.mult)
            GT = lt("gt")
            vec.tensor_scalar(out=GT[:], in0=PC[:], scalar1=IGNORE_HIGH_FP,
                              scalar2=None, op0=OP.is_lt)
            vec.tensor_tensor(out=T1[:], in0=T1[:], in1=GT[:], op=OP.mult)
            vec.tensor_tensor(out=T1[:], in0=T1[:], in1=VAL, op=OP.mult)
            NEGA = cp.tile([128, 1], F32)
            vec.tensor_reduce(out=NEGA[:], in_=T1[:], axis=AX.X, op=OP.add)
            # validity + rt
            VM = lt("vm")
            vec.tensor_scalar(out=VM[:], in0=MINWD[:, 1, :], scalar1=-INF / 2,
                              scalar2=None, op0=OP.is_gt)
            vec.tensor_tensor(out=VM[:], in0=VM[:], in1=VAL, op=OP.mult)
            REGC = cp.tile([128, 1], F32)
            vec.tensor_reduce(out=REGC[:], in_=VM[:], axis=AX.X, op=OP.add)
            # normalize one-hot sums by match count (ties averaged)
            RCPC = lt("rcpc")
            vec.reciprocal(out=RCPC[:], in_=XT5[:, 4, :])
            XT = wp.tile([128, 4, NT], F32, tag="xt", name="xt")
            vec.tensor_tensor(out=XT[:], in0=XT5[:, 0:4, :],
                              in1=RCPC[:].unsqueeze(1).broadcast_to((128, 4, NT)),
                              op=OP.mult)
            RT = wp.tile([128, 4, NT], F32, tag="rt", name="rt")
            vec.scalar_tensor_tensor(out=RT[:, 0:2, :], in0=XT[:, 0:2, :],
                                     scalar=-1.0, in1=SL[:, 0:2, :],
                                     op0=OP.mult, op1=OP.add)
            vec.scalar_tensor_tensor(out=RT[:, 2:4, :], in0=XT[:, 2:4, :],
                                     scalar=-1.0, in1=SL[:, 4:6, :],
                                     op0=OP.mult, op1=OP.add)
            # RT = signed_grid - XT = (l, t, r, b) of argmin box
            vec.tensor_tensor(out=RT[:], in0=RT[:],
                              in1=ISV.unsqueeze(1).broadcast_to((128, 4, NT)),
                              op=OP.mult)
            # rtf = rt*vm + (1-vm)   (exact select; vm in {0,1})
            RTF = wp.tile([128, 4, NT], F32, tag="rtf", name="rtf")
            vec.tensor_tensor(out=RTF[:], in0=RT[:],
                              in1=VM[:].unsqueeze(1).broadcast_to((128, 4, NT)),
                              op=OP.mult)
            VMN = lt("vmn")
            vec.tensor_scalar(out=VMN[:], in0=VM[:], scalar1=-1.0, scalar2=1.0,
                              op0=OP.mult, op1=OP.add)
            vec.tensor_tensor(out=RTF[:], in0=RTF[:],
                              in1=VMN[:].unsqueeze(1).broadcast_to((128, 4, NT)),
                              op=OP.add)
            # giou(pred, rtf)
            pl, pt = DY[:, 1, :], DY[:, 2, :]
            pr, pb = DY[:, 3, :], DY[:, 4, :]
            tl, tt_ = RTF[:, 0, :], RTF[:, 1, :]
            tr, tb = RTF[:, 2, :], RTF[:, 3, :]
            TA, PA, WI, GW, HI, GH = (lt("ta"), lt("pa"), lt("wi"), lt("gw"),
                                      lt("hi"), lt("gh"))
            SA, SB = lt("sa"), lt("sb")
            vec.tensor_tensor(out=SA[:], in0=tl, in1=tr, op=OP.add)
            vec.tensor_tensor(out=SB[:], in0=tt_, in1=tb, op=OP.add)
            vec.tensor_tensor(out=TA[:], in0=SA[:], in1=SB[:], op=OP.mult)
            vec.tensor_tensor(out=SA[:], in0=pl, in1=pr, op=OP.add)
            vec.tensor_tensor(out=SB[:], in0=pt, in1=pb, op=OP.add)
            vec.tensor_tensor(out=PA[:], in0=SA[:], in1=SB[:], op=OP.mult)
            vec.tensor_tensor(out=SA[:], in0=pl, in1=tl, op=OP.min)
            vec.tensor_tensor(out=SB[:], in0=pr, in1=tr, op=OP.min)
            vec.tensor_tensor(out=WI[:], in0=SA[:], in1=SB[:], op=OP.add)
            vec.tensor_tensor(out=SA[:], in0=pl, in1=tl, op=OP.max)
            vec.tensor_tensor(out=SB[:], in0=pr, in1=tr, op=OP.max)
            vec.tensor_tensor(out=GW[:], in0=SA[:], in1=SB[:], op=OP.add)
            vec.tensor_tensor(out=SA[:], in0=pb, in1=tb, op=OP.min)
            vec.tensor_tensor(out=SB[:], in0=pt, in1=tt_, op=OP.min)
            vec.tensor_tensor(out=HI[:], in0=SA[:], in1=SB[:], op=OP.add)
            vec.tensor_tensor(out=SA[:], in0=pb, in1=tb, op=OP.max)
            vec.tensor_tensor(out=SB[:], in0=pt, in1=tt_, op=OP.max)
            vec.tensor_tensor(out=GH[:], in0=SA[:], in1=SB[:], op=OP.add)
            AC = lt("ac")
            vec.tensor_tensor(out=AC[:], in0=GW[:], in1=GH[:], op=OP.mult)
            vec.tensor_scalar(out=AC[:], in0=AC[:], scalar1=EPS_AC,
                              scalar2=None, op0=OP.add)
            INTER = lt("inter")
            vec.tensor_tensor(out=INTER[:], in0=WI[:], in1=HI[:], op=OP.mult)
            UN = lt("un")
            vec.tensor_tensor(out=UN[:], in0=TA[:], in1=PA[:], op=OP.add)
            vec.tensor_tensor(out=UN[:], in0=UN[:], in1=INTER[:], op=OP.subtract)
            vec.tensor_scalar(out=SA[:], in0=INTER[:], scalar1=1.0,
                              scalar2=None, op0=OP.add)
            vec.tensor_scalar(out=SB[:], in0=UN[:], scalar1=1.0,
                              scalar2=None, op0=OP.add)
            IOU = lt("iou")
            vec.reciprocal(out=SB[:], in_=SB[:])
            vec.tensor_tensor(out=IOU[:], in0=SA[:], in1=SB[:], op=OP.mult)
            vec.tensor_tensor(out=SA[:], in0=AC[:], in1=UN[:], op=OP.subtract)
            vec.reciprocal(out=SB[:], in_=AC[:])
            vec.tensor_tensor(out=SB[:], in0=SA[:], in1=SB[:], op=OP.mult)
            vec.tensor_tensor(out=IOU[:], in0=IOU[:], in1=SB[:], op=OP.subtract)
            vec.tensor_scalar(out=IOU[:], in0=IOU[:], scalar1=-1.0, scalar2=1.0,
                              op0=OP.mult, op1=OP.add)  # 1 - giou
            vec.tensor_tensor(out=IOU[:], in0=IOU[:], in1=VM[:], op=OP.mult)
            REGA = cp.tile([128, 1], F32)
            vec.tensor_reduce(out=REGA[:], in_=IOU[:], axis=AX.X, op=OP.add)

            # ---------------- partial reduction (host finishes) -------------
            PART = cp.tile([128, 8], F32)
            vec.memset(PART[:], 0.0)
            vec.tensor_copy(out=PART[:, 0:1], in_=REGA[:])
            vec.tensor_copy(out=PART[:, 1:2], in_=REGC[:])
            vec.tensor_copy(out=PART[:, 3:4], in_=NEGA[:])
            vec.tensor_copy(out=PART[0:NBOX, 2:3], in_=POSS[:])
            vec.tensor_copy(out=PART[0:NBOX, 4:5], in_=NPOS[:])
            ONES = cp.tile([128, 1], F32)
            vec.memset(ONES[:], 1.0)
            PS = pp.tile([1, 8], F32, bufs=1)
            nc.tensor.matmul(out=PS[:], lhsT=ONES[:], rhs=PART[:],
                             start=True, stop=True)
            PSB = cp.tile([1, 8], F32)
            vec.tensor_copy(out=PSB[:], in_=PS[:])
            if dbg:
                sync.dma_start(out=minddbg[:], in_=MINWD[:, 1, :])
                sync.dma_start(out=minwdbg[:], in_=MINWD[:, 0, :])
                sync.dma_start(out=xtdbg[:], in_=XT5[:, 0:4, :])
                sync.dma_start(out=posdbg[:], in_=POSF[:])
                sync.dma_start(out=gvdbg[:], in_=GV[:])
            sync.dma_start(out=out[:], in_=PSB[:])
    nc.compile()
    return nc


# ------------------------------ host wrapper -------------------------------

def make_in_maps(boxes, agn_hm_pred, reg_pred):
    boxes = np.ascontiguousarray(np.asarray(boxes, np.float32))
    agn = np.ascontiguousarray(np.asarray(agn_hm_pred, np.float32))
    rp = np.ascontiguousarray(np.asarray(reg_pred, np.float32))
    agnfull = np.ascontiguousarray(agn.reshape(M_TOT, 1))
    in_maps = []
    for c in range(N_CORES):
        b, h = c // 2, c % 2
        idx = _SHARD_IDX[(b, h)]
        dyn = np.zeros((128, 5, NT), np.float32)
        a = np.zeros(NPAD, np.float32)
        a[:NV] = agn[idx]
        dyn[:, 0, :] = _pack(a)
        r = np.zeros((NPAD, 4), np.float32)
        r[:NV] = rp[idx]
        for k in range(4):
            dyn[:, 1 + k, :] = _pack(np.ascontiguousarray(r[:, k]))
        corec = np.zeros((NBOX, 8), np.float32)
        for l in range(5):
            corec[:, l] = BASE[l] + b * LOC[l]
        corec[:, 5] = 1.0 if h == 0 else 0.0
        in_maps.append({
            "locst": _LOCSTAT[h],
            "dyn": np.ascontiguousarray(dyn),
            "boxesT": np.ascontiguousarray(boxes[b].T),
            "boxesP": np.ascontiguousarray(np.tile(boxes[b], (2, 1))),
            "agnfull": agnfull,
            "corec": corec,
            "lvlc": _LVLC,
        })
    return in_maps


_NC_CACHE = {}
LAST_RESULT = None


def _get_nc():
    if "nc" not in _NC_CACHE:
        _NC_CACHE["nc"] = build_nc(dbg=False)
    return _NC_CACHE["nc"]


def kernel(boxes, gt_classes=None, agn_hm_pred=None, reg_pred=None):
    global LAST_RESULT
    in_maps = make_in_maps(boxes, agn_hm_pred, reg_pred)
    nc = _get_nc()
    res = run_bass_kernel_spmd(nc, in_maps, core_ids=list(range(N_CORES)))
    LAST_RESULT = res
    red = np.zeros(8, np.float64)
    for r in res.results:
        red += np.asarray(r["out"], np.float32).reshape(8).astype(np.float64)
    rega, regc, poss, nega, npos = red[0], red[1], red[2], red[3], red[4]
    npa = max(npos, 1.0)
    reg_loss = rega / max(regc, 1.0)
    agn_pos = -0.125 * poss / npa
    agn_neg = -0.375 * nega / npa
    return np.array([reg_loss, agn_pos, agn_neg], np.float32)



# revision 71
# speedup vs baseline: 1.0790x; 1.0790x over previous
"""CenterNet loss (GT assignment + focal/giou losses) on 8 Trainium2 cores.

Sharding: core c handles image b = c//2 and half h = c%2 of EVERY FPN level
(so all 8 cores run an identical SPMD tile schedule). Each core produces
partial sums (giou_sum, reg_cnt, pos_sum, neg_sum, npos); a DRAM AllReduce
combines them and every core computes the final 3-vector.
"""

import numpy as np

import concourse.bass as bass
import concourse.bacc as bacc
import concourse.tile as tile
from concourse import ap_utils, mybir
from concourse.bass_utils import run_bass_kernel_spmd


def _pool_on(eng, nc, out, in_, func):
    """Emit InstPool (innermost-dim reduction) on the given engine.

    Pads the input AP to 5-D (hardware requirement) via unsqueeze."""
    while len(in_.shape) < 5:
        in_ = in_.unsqueeze(1)
    return eng.add_instruction(mybir.InstPool(
        name=f"I-{nc.next_id()}", func=func,
        ins=[eng.lower_ap(in_, opt=False)], outs=[eng.lower_ap(out)]))

F32 = mybir.dt.float32
I32 = mybir.dt.int32
AF = mybir.ActivationFunctionType
OP = mybir.AluOpType
AX = mybir.AxisListType

# ---------------- problem constants (hardcoded from the nn.Module) ---------
B, NBOX = 4, 64
STRIDES = (8, 16, 32, 64, 128)
LEVEL_HW = ((128, 128), (64, 64), (32, 32), (16, 16), (8, 8))
SIZES = ((0.0, 80.0), (64.0, 160.0), (128.0, 320.0), (256.0, 640.0), (512.0, 1e7))
LOC = [h * w for h, w in LEVEL_HW]          # [16384, 4096, 1024, 256, 64]
M_IMG = sum(LOC)                            # 21824
M_TOT = B * M_IMG                           # 87296
BASE = [0, 65536, 81920, 86016, 87040]      # global level bases (level-major)
HALF = [m // 2 for m in LOC]                # per-core per-level loc counts
NT = 86                                     # 128-loc tiles per core
NV = sum(HALF)                              # 10912 valid locs per core
NPAD = NT * 128                             # 11008
INF = 1e8
MIN_RADIUS2 = 16.0
DELTA = (1 - 0.8) / (1 + 0.8)
K_R2 = float(np.float32(DELTA ** 2 * 2))    # radius2 = max(K_R2*area, 16)
SIG_LO = float(np.float32(1e-4))
SIG_HI = float(np.float32(1.0 - 1e-4))
EPS_AC = float(np.float32(1e-7))
IGNORE_HIGH_FP = 0.85
MAGIC = 8388608.0  # 2^23: u+MAGIC-MAGIC rounds u to nearest int (u < 2^22)
M15 = 12582912.0   # 1.5*2^23: (u+(M15-.5))-M15 = floor(u) for non-half-int u
# supergroups: (tile0, n_tiles, level); all tiles in a group share a level
SG = [(i * 16, 16, 0) for i in range(4)] + [
    (64, 16, 1), (80, 4, 2), (84, 1, 3), (85, 1, 4)]
# tiles per level: L0 t0-63, L1 64-79, L2 80-83, L3 84, L4 85 (32 valid rows)

N_CORES = 8


def _pack(vec):
    """[NPAD] (loc j = t*128+p) -> [128, NT] with [p, t] layout."""
    return np.ascontiguousarray(vec.reshape(NT, 128).T)


def _grids_per_level():
    gs = []
    for (h, w), s in zip(LEVEL_HW, STRIDES):
        ys, xs = np.meshgrid(np.arange(h) * s, np.arange(w) * s, indexing="ij")
        g = np.stack([xs.reshape(-1), ys.reshape(-1)], 1).astype(np.float32) + s // 2
        gs.append(g)
    return gs


def _half_concat(per_level_fn, h):
    """Concat per-level arrays for half h, pad to NPAD."""
    parts = [per_level_fn(l, h) for l in range(5)]
    cat = np.concatenate(parts, 0)
    pad_shape = (NPAD - NV,) + cat.shape[1:]
    return np.concatenate([cat, np.zeros(pad_shape, cat.dtype)], 0)


_GRIDS = _grids_per_level()


def _build_locstat(h):
    """[128, 8, NT]: planes gx, gy, gx, gy, -gx, -gy, valid, inv_s."""
    g = _half_concat(lambda l, hh: _GRIDS[l][hh * HALF[l]:(hh + 1) * HALF[l]], h)
    gx, gy = g[:, 0], g[:, 1]
    valid = np.zeros(NPAD, np.float32)
    valid[:NV] = 1.0
    inv_s = _half_concat(
        lambda l, hh: np.full(HALF[l], 1.0 / STRIDES[l], np.float32), h)
    inv_s[NV:] = 1.0
    planes = [gx, gy, gx, gy, -gx, -gy, valid, inv_s]
    out = np.stack([_pack(p.astype(np.float32)) for p in planes], 1)
    return np.ascontiguousarray(out)  # [128, 8, NT]


_LOCSTAT = [_build_locstat(0), _build_locstat(1)]


def _shard_idx(b, h):
    """Global level-major indices of core (b, h)'s NV locations."""
    parts = [BASE[l] + b * LOC[l] + h * HALF[l] + np.arange(HALF[l])
             for l in range(5)]
    return np.concatenate(parts, 0)


_SHARD_IDX = {(b, h): _shard_idx(b, h) for b in range(B) for h in range(2)}

# per-level constants [128, 8, 5]:
# inv_s, s, s/2, lo, hi, W, 4*lo^2, 4*hi^2 (squared-domain cared tests)
_LVLC = np.ascontiguousarray(np.broadcast_to(np.stack([
    np.array([1.0 / s for s in STRIDES], np.float32),
    np.array(STRIDES, np.float32),
    np.array([s / 2.0 for s in STRIDES], np.float32),
    np.array([r[0] for r in SIZES], np.float32),
    np.array([r[1] for r in SIZES], np.float32),
    np.array([w for (_, w) in LEVEL_HW], np.float32),
    np.array([4.0 * r[0] * r[0] for r in SIZES], np.float32),
    np.array([4.0 * r[1] * r[1] for r in SIZES], np.float32),
], 0), (128, 8, 5)).astype(np.float32))


# ------------------------------ device program -----------------------------

def build_nc(with_cc=False, dbg=False):
    nc = bacc.Bacc(trn_type="TRN2", num_devices=N_CORES)
    locst = nc.dram_tensor("locst", [128, 8, NT], F32, kind="ExternalInput")
    dyn = nc.dram_tensor("dyn", [128, 5, NT], F32, kind="ExternalInput")
    boxesT = nc.dram_tensor("boxesT", [4, NBOX], F32, kind="ExternalInput")
    boxesP = nc.dram_tensor("boxesP", [2 * NBOX, 4], F32, kind="ExternalInput")
    lvlc = nc.dram_tensor("lvlc", [128, 8, 5], F32, kind="ExternalInput")
    out = nc.dram_tensor("out", [1, 8], F32, kind="ExternalOutput")
    if dbg:
        minddbg = nc.dram_tensor("minddbg", [128, NT], F32, kind="ExternalOutput")
        minwdbg = nc.dram_tensor("minwdbg", [128, NT], F32, kind="ExternalOutput")
        xtdbg = nc.dram_tensor("xtdbg", [128, 4, NT], F32, kind="ExternalOutput")
        posdbg = nc.dram_tensor("posdbg", [NBOX, 5], F32, kind="ExternalOutput")
        gvdbg = nc.dram_tensor("gvdbg", [NBOX, 5], F32, kind="ExternalOutput")
    vec, act, gps, sync = nc.vector, nc.scalar, nc.gpsimd, nc.sync

    with tile.TileContext(nc) as tc:
        with tc.tile_pool(name="const", bufs=1) as cp, \
             tc.tile_pool(name="work", bufs=3) as wp, \
             tc.tile_pool(name="ppool", bufs=3, space="PSUM") as pp:

            # ---------------- loads ----------------
            SL = cp.tile([128, 8, NT], F32)
            sync.dma_start(out=SL[:], in_=locst[:])
            DY = cp.tile([128, 5, NT], F32)
            sync.dma_start(out=DY[:], in_=dyn[:])
            BBh = boxesT[:]
            BB = cp.tile([128, 4, NBOX], F32)
            bt_bc = bass.AP(tensor=BBh.tensor, offset=BBh.offset,
                            ap=[[0, 128], [NBOX, 4], [1, NBOX]])
            sync.dma_start(out=BB[:], in_=bt_bc)
            BP = cp.tile([2 * NBOX, 4], F32)
            sync.dma_start(out=BP[:], in_=boxesP[:])
            LV = cp.tile([128, 8, 5], F32)
            sync.dma_start(out=LV[:], in_=lvlc[:])
            # ---------------- per-box precompute ([128, 64] broadcast) -----
            from concourse.masks import make_identity
            IDT = cp.tile([128, 128], mybir.dt.bfloat16)
            make_identity(nc, IDT[:])
            x0, y0 = BB[:, 0, :], BB[:, 1, :]
            x1, y1 = BB[:, 2, :], BB[:, 3, :]
            CXY = cp.tile([128, 2, NBOX], F32)
            CX, CY = CXY[:, 0, :], CXY[:, 1, :]
            vec.tensor_tensor(out=CXY[:], in0=BB[:, 0:2, :], in1=BB[:, 2:4, :],
                              op=OP.add)
            vec.tensor_scalar(out=CXY[:], in0=CXY[:], scalar1=0.5, scalar2=None,
                              op0=OP.mult)
            # W2C = (w/2, h/2) per box; W2SQ = squared (bf16)
            W2C = cp.tile([128, 2, NBOX], F32)
            vec.tensor_tensor(out=W2C[:], in0=BB[:, 2:4, :], in1=BB[:, 0:2, :],
                              op=OP.subtract)
            vec.tensor_scalar(out=W2C[:], in0=W2C[:], scalar1=0.5, scalar2=None,
                              op0=OP.mult)
            W2SQ = cp.tile([128, 2, NBOX], mybir.dt.bfloat16)
            act.square(out=W2SQ[:], in_=W2C[:])
            S1 = cp.tile([128, NBOX], F32)
            S2 = cp.tile([128, NBOX], F32)
            # radius2 = max(K_R2 * area, 16);  IR2N = -1/radius2
            R2 = cp.tile([128, NBOX], F32)
            vec.tensor_tensor(out=S1[:], in0=x1, in1=x0, op=OP.subtract)  # w
            vec.tensor_tensor(out=S2[:], in0=y1, in1=y0, op=OP.subtract)  # h
            vec.tensor_tensor(out=R2[:], in0=S1[:], in1=S2[:], op=OP.mult)
            vec.tensor_scalar(out=R2[:], in0=R2[:], scalar1=K_R2,
                              scalar2=MIN_RADIUS2, op0=OP.mult, op1=OP.max)
            IR2N = cp.tile([128, NBOX], F32)
            vec.reciprocal(out=IR2N[:], in_=R2[:])
            vec.tensor_scalar(out=IR2N[:], in0=IR2N[:], scalar1=-1.0,
                              scalar2=None, op0=OP.mult)
            IR2NB = cp.tile([128, NBOX], mybir.dt.bfloat16)
            vec.tensor_copy(out=IR2NB[:], in_=IR2N[:])
            # S1 = w^2 + h^2 = (2*crit)^2 (cared tests done in squared domain)
            vec.tensor_tensor(out=S1[:], in0=S1[:], in1=S1[:], op=OP.mult)
            vec.tensor_tensor(out=S2[:], in0=S2[:], in1=S2[:], op=OP.mult)
            vec.tensor_tensor(out=S1[:], in0=S1[:], in1=S2[:], op=OP.add)
            # PBS [128, 4] = (x0, y0, -x1, -y1) boxes-on-partitions
            PBS = cp.tile([2 * NBOX, 4], F32)
            vec.tensor_copy(out=PBS[:, 0:2], in_=BP[:, 0:2])
            vec.tensor_scalar(out=PBS[:, 2:4], in0=BP[:, 2:4], scalar1=-1.0,
                              scalar2=None, op0=OP.mult)
            # bf16 hi/lo split of PBS (hi+lo covers f32 to ~2^-17 rel).
            # 5th column: (1, 0) so the one-hot matmul also yields the
            # match count (ties are averaged via the count column).
            BF16 = mybir.dt.bfloat16
            NSPL = 2
            PBS3 = cp.tile([NBOX, NSPL, 5], BF16)
            PR1 = cp.tile([NBOX, 4], F32)
            vec.memset(PBS3[:], 0.0)
            vec.memset(PBS3[:, 0, 4:5], 1.0)
            vec.tensor_copy(out=PBS3[:, 0, 0:4], in_=PBS[0:NBOX, :])
            vec.tensor_copy(out=PR1[:], in_=PBS3[:, 0, 0:4])  # hi back to f32
            vec.tensor_tensor(out=PR1[:], in0=PBS[0:NBOX, :], in1=PR1[:],
                              op=OP.subtract)
            vec.tensor_copy(out=PBS3[:, 1, 0:4], in_=PR1[:])
            # block-diagonal variant for paired-tile extraction:
            # rows 0:64 -> cols 0:5, rows 64:128 -> cols 5:10
            PBSD = cp.tile([128, NSPL, 10], BF16)
            vec.memset(PBSD[:], 0.0)
            vec.memset(PBSD[0:NBOX, 0, 4:5], 1.0)
            vec.memset(PBSD[NBOX:2 * NBOX, 0, 9:10], 1.0)
            SPL = cp.tile([128, NSPL, 4], BF16)
            PRF = cp.tile([128, 4], F32)
            vec.tensor_copy(out=SPL[:, 0, :], in_=PBS[:])
            vec.tensor_copy(out=PRF[:], in_=SPL[:, 0, :])
            vec.tensor_tensor(out=PRF[:], in0=PBS[:], in1=PRF[:],
                              op=OP.subtract)
            vec.tensor_copy(out=SPL[:, 1, :], in_=PRF[:])
            vec.tensor_copy(out=PBSD[0:NBOX, :, 0:4], in_=SPL[0:NBOX, :, :])
            vec.tensor_copy(out=PBSD[NBOX:2 * NBOX, :, 5:9],
                            in_=SPL[NBOX:2 * NBOX, :, :])
            # bf16 grid copy (lattice-exact: multiples of 4 <= 1020)
            BF16 = mybir.dt.bfloat16
            SLB = cp.tile([128, 2, NT], BF16)
            vec.tensor_copy(out=SLB[:], in_=SL[:, 0:2, :])
            # batched over levels: BCQ[l] = cared ? -1 : >=1 (bf16, max-combine);
            # CDALL = (cdisx, cdisy) per level, bf16 lattice-exact
            BCQ = []
            SF1 = cp.tile([128, NBOX], F32, tag="sf1", name="sf1")
            for l in range(5):
                lo, hi = SIZES[l]
                lo2, hi2 = 4.0 * lo * lo, 4.0 * hi * hi
                cn = cp.tile([128, NBOX], BF16, tag=f"bcq{l}", name=f"bcq{l}")
                vec.tensor_scalar(out=SF1[:], in0=S1[:], scalar1=lo2,
                                  scalar2=None, op0=OP.is_lt)
                vec.scalar_tensor_tensor(out=SF1[:], in0=S1[:],
                                         scalar=hi2, in1=SF1[:],
                                         op0=OP.is_gt, op1=OP.add)
                vec.tensor_scalar(out=cn[:], in0=SF1[:], scalar1=2.0,
                                  scalar2=-1.0, op0=OP.mult, op1=OP.add)
                BCQ.append(cn)

            def lvb(pl, shape):  # LV plane [128,5] -> bcast (128, d1, 5, 64)
                return (LV[:, pl, :].unsqueeze(1).broadcast_to((128, shape[1], 5))
                        .unsqueeze(3).broadcast_to(shape))

            B254 = (128, 2, 5, 64)
            UU = cp.tile([128, 2, 5, 64], F32, tag="uu", name="uu")
            R0 = cp.tile([128, 2, 5, 64], F32, tag="r0", name="r0")
            CC = cp.tile([128, 2, 5, 64], F32, tag="ccf", name="ccf")
            vec.tensor_tensor(
                out=UU[:], in0=CXY[:].unsqueeze(2).broadcast_to(B254),
                in1=lvb(0, B254), op=OP.mult)  # u = c/s
            vec.tensor_scalar(out=R0[:], in0=UU[:], scalar1=MAGIC,
                              scalar2=MAGIC, op0=OP.add, op1=OP.subtract)
            vec.tensor_tensor(out=CC[:], in0=R0[:], in1=UU[:], op=OP.is_gt)
            vec.tensor_tensor(out=R0[:], in0=R0[:], in1=CC[:], op=OP.subtract)
            CDALL = cp.tile([128, 2, 5, 64], BF16, tag="cdall", name="cdall")
            vec.tensor_tensor(out=R0[:], in0=R0[:], in1=lvb(1, B254),
                              op=OP.mult)
            vec.tensor_tensor(out=CDALL[:], in0=R0[:], in1=lvb(2, B254),
                              op=OP.add)  # floor(c/s)*s + s/2

            # ---------------- main pair loop --------------------------------
            # negated-min convention: plane 0 = -min(wdist2), 1 = -min(d)
            MINWD = cp.tile([128, 2, NT], BF16)
            # selected (x0, y0, -x1, -y1, count) sums over argmin one-hot
            XT5 = cp.tile([128, 5, NT], F32)

            for (t0, G, l) in SG:
                s = float(STRIDES[l])
                s2 = float(s * s)
                sl = slice(t0, t0 + G)

                def bb1(t2d):  # [128,64] const -> [128,G,64]
                    return t2d.unsqueeze(1).broadcast_to((128, G, 64))

                # DFC = (gx-cx, gy-cy) fp32
                DFC = wp.tile([128, 2, G, 64], F32, tag="dfc", name="dfc")
                vec.tensor_tensor(
                    out=DFC[:],
                    in0=SL[:, 0:2, sl].unsqueeze(3).broadcast_to((128, 2, G, 64)),
                    in1=CXY[:].unsqueeze(2).broadcast_to((128, 2, G, 64)),
                    op=OP.subtract)
                # DFD = (gx-cdisx, gy-cdisy) bf16, lattice-exact
                DFD = wp.tile([128, 2, G, 64], BF16, tag="dfd", name="dfd")
                vec.tensor_tensor(
                    out=DFD[:],
                    in0=SLB[:, :, sl].unsqueeze(3).broadcast_to((128, 2, G, 64)),
                    in1=CDALL[:, :, l, :].unsqueeze(2).broadcast_to(
                        (128, 2, G, 64)),
                    op=OP.subtract)
                SQ2 = wp.tile([128, 2, G, 64], BF16, tag="sq2", name="sq2")
                act.square(out=SQ2[:], in_=DFC[:])
                SQD = wp.tile([128, 2, G, 64], BF16, tag="sqd", name="sqd")
                act.square(out=SQD[:], in_=DFD[:])
                # in-box test in squared domain: dfx^2 >= (w/2)^2 -> outside
                MXQ = wp.tile([128, 2, G, 64], BF16, tag="mxq", name="mxq")
                vec.tensor_tensor(
                    out=MXQ[:], in0=SQ2[:],
                    in1=W2SQ[:].unsqueeze(2).broadcast_to((128, 2, G, 64)),
                    op=OP.subtract)
                M4 = wp.tile([128, G, 64], BF16, tag="m4", name="m4")
                vec.tensor_tensor(out=M4[:], in0=MXQ[:, 0], in1=MXQ[:, 1],
                                  op=OP.max)
                # peak / 3x3 tests on squared bf16 lattice values
                MQ = wp.tile([128, G, 64], BF16, tag="mq", name="mq")
                vec.tensor_tensor(out=MQ[:], in0=SQD[:, 0], in1=SQD[:, 1],
                                  op=OP.max)
                # invalid <=> max(MQ - 1.5*s^2, M4, BCQ) >= 0 (3x3 inclusive:
                # MQ lattice jumps s^2 -> 4s^2, so -1.5s^2 keeps MQ==s^2 valid)
                V = wp.tile([128, G, 64], BF16, tag="vv", name="vv")
                vec.scalar_tensor_tensor(out=V[:], in0=MQ[:],
                                         scalar=-1.5 * s2, in1=M4[:],
                                         op0=OP.add, op1=OP.max)
                vec.tensor_tensor(out=V[:], in0=V[:], in1=bb1(BCQ[l][:]),
                                  op=OP.max)
                PEN = wp.tile([128, G, 64], BF16, tag="pen", name="pen")
                vec.tensor_scalar(out=PEN[:], in0=V[:], scalar1=0.0,
                                  scalar2=-INF, op0=OP.is_ge, op1=OP.mult)
                D2 = wp.tile([128, G, 64], BF16, tag="d2", name="d2")
                vec.tensor_tensor(out=D2[:], in0=SQ2[:, 0], in1=SQ2[:, 1],
                                  op=OP.add)
                DZ = wp.tile([128, G, 64], BF16, tag="dz", name="dz")
                vec.scalar_tensor_tensor(out=DZ[:], in0=MQ[:], scalar=0.0,
                                         in1=D2[:], op0=OP.not_equal,
                                         op1=OP.mult)
                # WDN/DN share one tile -> single paired reduce
                WDD = wp.tile([128, 2, G, 64], BF16, tag="wdd", name="wdd")
                vec.tensor_tensor(out=WDD[:, 0], in0=DZ[:], in1=bb1(IR2NB[:]),
                                  op=OP.mult)  # -wdist2
                vec.tensor_tensor(out=WDD[:, 1], in0=WDD[:, 0], in1=PEN[:],
                                  op=OP.add)
                vec.tensor_reduce(out=MINWD[:, :, sl], in_=WDD[:], axis=AX.X,
                                  op=OP.max)
                # direct one-hot (ties summed; normalized later via count col)
                OH = wp.tile([128, G, 64], mybir.dt.bfloat16, tag="oh", name="oh")
                vec.tensor_tensor(out=OH[:], in0=WDD[:, 1],
                                  in1=MINWD[:, 1, sl].unsqueeze(2).broadcast_to(
                                      (128, G, 64)), op=OP.is_equal)
                # rt extraction on PE: paired-tile transpose + block-diag rhs
                if G % 2 == 0:
                    P2 = G // 2
                    OHT = pp.tile([128, P2, 128], mybir.dt.bfloat16, tag="oht",
                                  name="oht")
                    for gg in range(P2):
                        nc.tensor.transpose(
                            OHT[:, gg, :],
                            OH[:, 2 * gg:2 * gg + 2, :].rearrange("p a b -> p (a b)"), IDT[:])
                    OHTS = wp.tile([128, P2, 128], mybir.dt.bfloat16,
                                   tag="ohts", name="ohts")
                    act.copy(out=OHTS[:], in_=OHT[:])
                    RTP = pp.tile([128, G, 5], F32, tag="rtp", name="rtp")
                    for gg in range(P2):
                        for k in range(NSPL):
                            nc.tensor.matmul(
                                out=RTP[:, 2 * gg:2 * gg + 2, :].rearrange("p a b -> p (a b)"),
                                lhsT=OHTS[:, gg, :], rhs=PBSD[:, k, :],
                                start=(k == 0), stop=(k == NSPL - 1))
                else:
                    OHT = pp.tile([64, G, 128], mybir.dt.bfloat16, tag="oht1",
                                  name="oht1", bufs=1)
                    for g in range(G):
                        nc.tensor.transpose(OHT[:, g, :], OH[:, g, :], IDT[:])
                    OHTS = wp.tile([64, G, 128], mybir.dt.bfloat16,
                                   tag="ohts1", name="ohts1")
                    act.copy(out=OHTS[:], in_=OHT[:])
                    RTP = pp.tile([128, G, 5], F32, tag="rtp", name="rtp")
                    for g in range(G):
                        for k in range(NSPL):
                            nc.tensor.matmul(out=RTP[:, g, :],
                                             lhsT=OHTS[:, g, :],
                                             rhs=PBS3[:, k, :],
                                             start=(k == 0),
                                             stop=(k == NSPL - 1))
                act.copy(out=XT5[:, :, sl], in_=RTP[:].transpose([0, 2, 1]))

            # ---------------- epilogue: per-location [128, NT] --------------
            AGN = DY[:, 0, :]
            VAL = SL[:, 6, :]
            ISV = SL[:, 7, :]

            def lt(tag):
                return wp.tile([128, NT], F32, tag=tag, name=tag)

            HM = lt("hm")
            act.activation(out=HM[:], in_=MINWD[:, 0, :], func=AF.Exp, scale=1.0)
            vec.scalar_tensor_tensor(out=HM[:], in0=HM[:], scalar=SIG_LO,
                                     in1=HM[:], op0=OP.is_ge, op1=OP.mult)
            NW = lt("nw")
            vec.tensor_scalar(out=NW[:], in0=HM[:], scalar1=-1.0, scalar2=1.0,
                              op0=OP.mult, op1=OP.add)
            vec.tensor_tensor(out=NW[:], in0=NW[:], in1=NW[:], op=OP.mult)
            vec.tensor_tensor(out=NW[:], in0=NW[:], in1=NW[:], op=OP.mult)
            PC = lt("pc")
            act.activation(out=PC[:], in_=AGN, func=AF.Sigmoid)
            vec.tensor_scalar(out=PC[:], in0=PC[:], scalar1=SIG_LO,
                              scalar2=SIG_HI, op0=OP.max, op1=OP.min)
            Q = lt("q")
            vec.tensor_scalar(out=Q[:], in0=PC[:], scalar1=-1.0, scalar2=1.0,
                              op0=OP.mult, op1=OP.add)
            act.activation(out=Q[:], in_=Q[:], func=AF.Ln)  # log(1-pred)
            P2 = lt("p2")
            vec.tensor_tensor(out=P2[:], in0=PC[:], in1=PC[:], op=OP.mult)
            T1 = lt("t1")
            vec.tensor_tensor(out=T1[:], in0=Q[:], in1=P2[:], op=OP.mult)
            vec.tensor_tensor(out=T1[:], in0=T1[:], in1=NW[:], op=OP.mult)
            GT = lt("gt")
            vec.tensor_scalar(out=GT[:], in0=PC[:], scalar1=IGNORE_HIGH_FP,
                              scalar2=None, op0=OP.is_lt)
            vec.tensor_tensor(out=T1[:], in0=T1[:], in1=GT[:], op=OP.mult)
            vec.tensor_tensor(out=T1[:], in0=T1[:], in1=VAL, op=OP.mult)
            NEGA = cp.tile([128, 1], F32)
            vec.tensor_reduce(out=NEGA[:], in_=T1[:], axis=AX.X, op=OP.add)
            # validity + rt
            VM = lt("vm")
            vec.tensor_scalar(out=VM[:], in0=MINWD[:, 1, :], scalar1=-INF / 2,
                              scalar2=None, op0=OP.is_gt)
            vec.tensor_tensor(out=VM[:], in0=VM[:], in1=VAL, op=OP.mult)
            REGC = cp.tile([128, 1], F32)
            vec.tensor_reduce(out=REGC[:], in_=VM[:], axis=AX.X, op=OP.add)
            # normalize one-hot sums by match count (ties averaged)
            RCPC = lt("rcpc")
            vec.reciprocal(out=RCPC[:], in_=XT5[:, 4, :])
            XT = wp.tile([128, 4, NT], F32, tag="xt", name="xt")
            vec.tensor_tensor(out=XT[:], in0=XT5[:, 0:4, :],
                              in1=RCPC[:].unsqueeze(1).broadcast_to((128, 4, NT)),
                              op=OP.mult)
            RT = wp.tile([128, 4, NT], F32, tag="rt", name="rt")
            vec.scalar_tensor_tensor(out=RT[:, 0:2, :], in0=XT[:, 0:2, :],
                                     scalar=-1.0, in1=SL[:, 0:2, :],
                                     op0=OP.mult, op1=OP.add)
            vec.scalar_tensor_tensor(out=RT[:, 2:4, :], in0=XT[:, 2:4, :],
                                     scalar=-1.0, in1=SL[:, 4:6, :],
                                     op0=OP.mult, op1=OP.add)
            # RT = signed_grid - XT = (l, t, r, b) of argmin box
            vec.tensor_tensor(out=RT[:], in0=RT[:],
                              in1=ISV.unsqueeze(1).broadcast_to((128, 4, NT)),
                              op=OP.mult)
            # rtf = rt*vm + (1-vm)   (exact select; vm in {0,1})
            RTF = wp.tile([128, 4, NT], F32, tag="rtf", name="rtf")
            vec.tensor_tensor(out=RTF[:], in0=RT[:],
                              in1=VM[:].unsqueeze(1).broadcast_to((128, 4, NT)),
                              op=OP.mult)
            VMN = lt("vmn")
            vec.tensor_scalar(out=VMN[:], in0=VM[:], scalar1=-1.0, scalar2=1.0,
                              op0=OP.mult, op1=OP.add)
            vec.tensor_tensor(out=RTF[:], in0=RTF[:],
                              in1=VMN[:].unsqueeze(1).broadcast_to((128, 4, NT)),
                              op=OP.add)
            # giou(pred, rtf)
            pl, pt = DY[:, 1, :], DY[:, 2, :]
            pr, pb = DY[:, 3, :], DY[:, 4, :]
            tl, tt_ = RTF[:, 0, :], RTF[:, 1, :]
            tr, tb = RTF[:, 2, :], RTF[:, 3, :]
            TA, PA, WI, GW, HI, GH = (lt("ta"), lt("pa"), lt("wi"), lt("gw"),
                                      lt("hi"), lt("gh"))
            SA, SB = lt("sa"), lt("sb")
            vec.tensor_tensor(out=SA[:], in0=tl, in1=tr, op=OP.add)
            vec.tensor_tensor(out=SB[:], in0=tt_, in1=tb, op=OP.add)
            vec.tensor_tensor(out=TA[:], in0=SA[:], in1=SB[:], op=OP.mult)
            vec.tensor_tensor(out=SA[:], in0=pl, in1=pr, op=OP.add)
            vec.tensor_tensor(out=SB[:], in0=pt, in1=pb, op=OP.add)
            vec.tensor_tensor(out=PA[:], in0=SA[:], in1=SB[:], op=OP.mult)
            vec.tensor_tensor(out=SA[:], in0=pl, in1=tl, op=OP.min)
            vec.tensor_tensor(out=SB[:], in0=pr, in1=tr, op=OP.min)
            vec.tensor_tensor(out=WI[:], in0=SA[:], in1=SB[:], op=OP.add)
            vec.tensor_tensor(out=SA[:], in0=pl, in1=tl, op=OP.max)
            vec.tensor_tensor(out=SB[:], in0=pr, in1=tr, op=OP.max)
            vec.tensor_tensor(out=GW[:], in0=SA[:], in1=SB[:], op=OP.add)
            vec.tensor_tensor(out=SA[:], in0=pb, in1=tb, op=OP.min)
            vec.tensor_tensor(out=SB[:], in0=pt, in1=tt_, op=OP.min)
            vec.tensor_tensor(out=HI[:], in0=SA[:], in1=SB[:], op=OP.add)
            vec.tensor_tensor(out=SA[:], in0=pb, in1=tb, op=OP.max)
            vec.tensor_tensor(out=SB[:], in0=pt, in1=tt_, op=OP.max)
            vec.tensor_tensor(out=GH[:], in0=SA[:], in1=SB[:], op=OP.add)
            AC = lt("ac")
            vec.tensor_tensor(out=AC[:], in0=GW[:], in1=GH[:], op=OP.mult)
            vec.tensor_scalar(out=AC[:], in0=AC[:], scalar1=EPS_AC,
                              scalar2=None, op0=OP.add)
            INTER = lt("inter")
            vec.tensor_tensor(out=INTER[:], in0=WI[:], in1=HI[:], op=OP.mult)
            UN = lt("un")
            vec.tensor_tensor(out=UN[:], in0=TA[:], in1=PA[:], op=OP.add)
            vec.tensor_tensor(out=UN[:], in0=UN[:], in1=INTER[:], op=OP.subtract)
            vec.tensor_scalar(out=SA[:], in0=INTER[:], scalar1=1.0,
                              scalar2=None, op0=OP.add)
            vec.tensor_scalar(out=SB[:], in0=UN[:], scalar1=1.0,
                              scalar2=None, op0=OP.add)
            IOU = lt("iou")
            vec.reciprocal(out=SB[:], in_=SB[:])
            vec.tensor_tensor(out=IOU[:], in0=SA[:], in1=SB[:], op=OP.mult)
            vec.tensor_tensor(out=SA[:], in0=AC[:], in1=UN[:], op=OP.subtract)
            vec.reciprocal(out=SB[:], in_=AC[:])
            vec.tensor_tensor(out=SB[:], in0=SA[:], in1=SB[:], op=OP.mult)
            vec.tensor_tensor(out=IOU[:], in0=IOU[:], in1=SB[:], op=OP.subtract)
            vec.tensor_scalar(out=IOU[:], in0=IOU[:], scalar1=-1.0, scalar2=1.0,
                              op0=OP.mult, op1=OP.add)  # 1 - giou
            vec.tensor_tensor(out=IOU[:], in0=IOU[:], in1=VM[:], op=OP.mult)
            REGA = cp.tile([128, 1], F32)
            vec.tensor_reduce(out=REGA[:], in_=IOU[:], axis=AX.X, op=OP.add)

            # ---------------- partial reduction (host finishes) -------------
            PART = cp.tile([128, 8], F32)
            vec.memset(PART[:], 0.0)
            vec.tensor_copy(out=PART[:, 0:1], in_=REGA[:])
            vec.tensor_copy(out=PART[:, 1:2], in_=REGC[:])
            vec.tensor_copy(out=PART[:, 3:4], in_=NEGA[:])
            ONES = cp.tile([128, 1], F32)
            vec.memset(ONES[:], 1.0)
            PS = pp.tile([1, 8], F32, bufs=1)
            nc.tensor.matmul(out=PS[:], lhsT=ONES[:], rhs=PART[:],
                             start=True, stop=True)
            PSB = cp.tile([1, 8], F32)
            vec.tensor_copy(out=PSB[:], in_=PS[:])
            if dbg:
                sync.dma_start(out=minddbg[:], in_=MINWD[:, 1, :])
                sync.dma_start(out=minwdbg[:], in_=MINWD[:, 0, :])
                sync.dma_start(out=xtdbg[:], in_=XT5[:, 0:4, :])
            sync.dma_start(out=out[:], in_=PSB[:])
    nc.compile()
    return nc


# ------------------------------ host wrapper -------------------------------

def make_in_maps(boxes, agn_hm_pred, reg_pred):
    boxes = np.ascontiguousarray(np.asarray(boxes, np.float32))
    agn = np.ascontiguousarray(np.asarray(agn_hm_pred, np.float32))
    rp = np.ascontiguousarray(np.asarray(reg_pred, np.float32))
    in_maps = []
    for c in range(N_CORES):
        b, h = c // 2, c % 2
        idx = _SHARD_IDX[(b, h)]
        dyn = np.zeros((128, 5, NT), np.float32)
        a = np.zeros(NPAD, np.float32)
        a[:NV] = agn[idx]
        dyn[:, 0, :] = _pack(a)
        r = np.zeros((NPAD, 4), np.float32)
        r[:NV] = rp[idx]
        for k in range(4):
            dyn[:, 1 + k, :] = _pack(np.ascontiguousarray(r[:, k]))
        in_maps.append({
            "locst": _LOCSTAT[h],
            "dyn": np.ascontiguousarray(dyn),
            "boxesT": np.ascontiguousarray(boxes[b].T),
            "boxesP": np.ascontiguousarray(np.tile(boxes[b], (2, 1))),
            "lvlc": _LVLC,
        })
    return in_maps


_NC_CACHE = {}
LAST_RESULT = None


def _get_nc():
    if "nc" not in _NC_CACHE:
        _NC_CACHE["nc"] = build_nc(dbg=False)
    return _NC_CACHE["nc"]


def kernel(boxes, gt_classes=None, agn_hm_pred=None, reg_pred=None):
    global LAST_RESULT
    in_maps = make_in_maps(boxes, agn_hm_pred, reg_pred)
    nc = _get_nc()
    res = run_bass_kernel_spmd(nc, in_maps, core_ids=list(range(N_CORES)))
    LAST_RESULT = res
    red = np.zeros(8, np.float64)
    for r in res.results:
        red += np.asarray(r["out"], np.float32).reshape(8).astype(np.float64)
    rega, regc, nega = red[0], red[1], red[3]
    # positive-sample focal term on host (tiny: [B, N, 5] indexing prep)
    bx = np.asarray(boxes, np.float32)
    agn = np.asarray(agn_hm_pred, np.float32).reshape(-1)
    centers = (bx[..., :2] + bx[..., 2:]) / 2
    st = np.asarray(STRIDES, np.float32)
    ci = (centers[:, :, None, :] / st[None, None, :, None]).astype(np.int64)
    Ws = np.asarray([w for (_, w) in LEVEL_HW], np.int64)
    locv = np.asarray(LOC, np.int64)
    basev = np.asarray(BASE, np.int64)
    im = np.arange(B)[:, None, None]
    pos = (basev[None, None, :] + im * locv[None, None, :]
           + ci[..., 1] * Ws[None, None, :] + ci[..., 0])
    idx = np.clip(pos.reshape(-1), 0, M_TOT - 1)
    crit = np.sqrt(((bx[..., 2:] - bx[..., :2]) ** 2).sum(-1)) / 2
    lov = np.asarray([r[0] for r in SIZES])
    hiv = np.asarray([r[1] for r in SIZES])
    mask = ((crit[..., None] >= lov) & (crit[..., None] <= hiv)).reshape(-1)
    pred = np.clip(1.0 / (1.0 + np.exp(-agn[idx].astype(np.float64))),
                   1e-4, 1.0 - 1e-4)
    poss = float(np.where(mask, np.log(pred) * (1.0 - pred) ** 2, 0.0).sum())
    npa = max(float(mask.sum()), 1.0)
    reg_loss = rega / max(regc, 1.0)
    agn_pos = -0.125 * poss / npa
    agn_neg = -0.375 * nega / npa
    return np.array([reg_loss, agn_pos, agn_neg], np.float32)



# revision 74
# speedup vs baseline: 1.0964x; 1.0162x over previous
"""CenterNet loss (GT assignment + focal/giou losses) on 8 Trainium2 cores.

Sharding: core c handles image b = c//2 and half h = c%2 of EVERY FPN level
(so all 8 cores run an identical SPMD tile schedule). Each core produces
partial sums (giou_sum, reg_cnt, pos_sum, neg_sum, npos); a DRAM AllReduce
combines them and every core computes the final 3-vector.
"""

import numpy as np

import concourse.bass as bass
import concourse.bacc as bacc
import concourse.tile as tile
from concourse import ap_utils, mybir
from concourse.bass_utils import run_bass_kernel_spmd


def _pool_on(eng, nc, out, in_, func):
    """Emit InstPool (innermost-dim reduction) on the given engine.

    Pads the input AP to 5-D (hardware requirement) via unsqueeze."""
    while len(in_.shape) < 5:
        in_ = in_.unsqueeze(1)
    return eng.add_instruction(mybir.InstPool(
        name=f"I-{nc.next_id()}", func=func,
        ins=[eng.lower_ap(in_, opt=False)], outs=[eng.lower_ap(out)]))

F32 = mybir.dt.float32
I32 = mybir.dt.int32
AF = mybir.ActivationFunctionType
OP = mybir.AluOpType
AX = mybir.AxisListType

# ---------------- problem constants (hardcoded from the nn.Module) ---------
B, NBOX = 4, 64
STRIDES = (8, 16, 32, 64, 128)
LEVEL_HW = ((128, 128), (64, 64), (32, 32), (16, 16), (8, 8))
SIZES = ((0.0, 80.0), (64.0, 160.0), (128.0, 320.0), (256.0, 640.0), (512.0, 1e7))
LOC = [h * w for h, w in LEVEL_HW]          # [16384, 4096, 1024, 256, 64]
M_IMG = sum(LOC)                            # 21824
M_TOT = B * M_IMG                           # 87296
BASE = [0, 65536, 81920, 86016, 87040]      # global level bases (level-major)
HALF = [m // 2 for m in LOC]                # per-core per-level loc counts
NT = 86                                     # 128-loc tiles per core
NV = sum(HALF)                              # 10912 valid locs per core
NPAD = NT * 128                             # 11008
INF = 1e8
MIN_RADIUS2 = 16.0
DELTA = (1 - 0.8) / (1 + 0.8)
K_R2 = float(np.float32(DELTA ** 2 * 2))    # radius2 = max(K_R2*area, 16)
SIG_LO = float(np.float32(1e-4))
SIG_HI = float(np.float32(1.0 - 1e-4))
EPS_AC = float(np.float32(1e-7))
IGNORE_HIGH_FP = 0.85
MAGIC = 8388608.0  # 2^23: u+MAGIC-MAGIC rounds u to nearest int (u < 2^22)
M15 = 12582912.0   # 1.5*2^23: (u+(M15-.5))-M15 = floor(u) for non-half-int u
# supergroups: (tile0, n_tiles, level); all tiles in a group share a level
SG = [(0, 32, 0), (32, 32, 0), (64, 16, 1), (80, 4, 2), (84, 1, 3), (85, 1, 4)]
# tiles per level: L0 t0-63, L1 64-79, L2 80-83, L3 84, L4 85 (32 valid rows)

N_CORES = 8


def _pack(vec):
    """[NPAD] (loc j = t*128+p) -> [128, NT] with [p, t] layout."""
    return np.ascontiguousarray(vec.reshape(NT, 128).T)


def _grids_per_level():
    gs = []
    for (h, w), s in zip(LEVEL_HW, STRIDES):
        ys, xs = np.meshgrid(np.arange(h) * s, np.arange(w) * s, indexing="ij")
        g = np.stack([xs.reshape(-1), ys.reshape(-1)], 1).astype(np.float32) + s // 2
        gs.append(g)
    return gs


def _half_concat(per_level_fn, h):
    """Concat per-level arrays for half h, pad to NPAD."""
    parts = [per_level_fn(l, h) for l in range(5)]
    cat = np.concatenate(parts, 0)
    pad_shape = (NPAD - NV,) + cat.shape[1:]
    return np.concatenate([cat, np.zeros(pad_shape, cat.dtype)], 0)


_GRIDS = _grids_per_level()


def _build_locstat(h):
    """[128, 8, NT]: planes gx, gy, gx, gy, -gx, -gy, valid, inv_s."""
    g = _half_concat(lambda l, hh: _GRIDS[l][hh * HALF[l]:(hh + 1) * HALF[l]], h)
    gx, gy = g[:, 0], g[:, 1]
    valid = np.zeros(NPAD, np.float32)
    valid[:NV] = 1.0
    inv_s = _half_concat(
        lambda l, hh: np.full(HALF[l], 1.0 / STRIDES[l], np.float32), h)
    inv_s[NV:] = 1.0
    planes = [gx, gy, gx, gy, -gx, -gy, valid, inv_s]
    out = np.stack([_pack(p.astype(np.float32)) for p in planes], 1)
    return np.ascontiguousarray(out)  # [128, 8, NT]


_LOCSTAT = [_build_locstat(0), _build_locstat(1)]


def _shard_idx(b, h):
    """Global level-major indices of core (b, h)'s NV locations."""
    parts = [BASE[l] + b * LOC[l] + h * HALF[l] + np.arange(HALF[l])
             for l in range(5)]
    return np.concatenate(parts, 0)


_SHARD_IDX = {(b, h): _shard_idx(b, h) for b in range(B) for h in range(2)}

# per-level constants [128, 8, 5]:
# inv_s, s, s/2, lo, hi, W, 4*lo^2, 4*hi^2 (squared-domain cared tests)
_LVLC = np.ascontiguousarray(np.broadcast_to(np.stack([
    np.array([1.0 / s for s in STRIDES], np.float32),
    np.array(STRIDES, np.float32),
    np.array([s / 2.0 for s in STRIDES], np.float32),
    np.array([r[0] for r in SIZES], np.float32),
    np.array([r[1] for r in SIZES], np.float32),
    np.array([w for (_, w) in LEVEL_HW], np.float32),
    np.array([4.0 * r[0] * r[0] for r in SIZES], np.float32),
    np.array([4.0 * r[1] * r[1] for r in SIZES], np.float32),
], 0), (128, 8, 5)).astype(np.float32))


# ------------------------------ device program -----------------------------

def build_nc(with_cc=False, dbg=False):
    nc = bacc.Bacc(trn_type="TRN2", num_devices=N_CORES)
    locst = nc.dram_tensor("locst", [128, 8, NT], F32, kind="ExternalInput")
    dyn = nc.dram_tensor("dyn", [128, 5, NT], F32, kind="ExternalInput")
    boxesT = nc.dram_tensor("boxesT", [4, NBOX], F32, kind="ExternalInput")
    boxesP = nc.dram_tensor("boxesP", [2 * NBOX, 4], F32, kind="ExternalInput")
    lvlc = nc.dram_tensor("lvlc", [128, 8, 5], F32, kind="ExternalInput")
    out = nc.dram_tensor("out", [1, 8], F32, kind="ExternalOutput")
    if dbg:
        minddbg = nc.dram_tensor("minddbg", [128, NT], F32, kind="ExternalOutput")
        minwdbg = nc.dram_tensor("minwdbg", [128, NT], F32, kind="ExternalOutput")
        xtdbg = nc.dram_tensor("xtdbg", [128, 4, NT], F32, kind="ExternalOutput")
        posdbg = nc.dram_tensor("posdbg", [NBOX, 5], F32, kind="ExternalOutput")
        gvdbg = nc.dram_tensor("gvdbg", [NBOX, 5], F32, kind="ExternalOutput")
    vec, act, gps, sync = nc.vector, nc.scalar, nc.gpsimd, nc.sync

    with tile.TileContext(nc) as tc:
        with tc.tile_pool(name="const", bufs=1) as cp, \
             tc.tile_pool(name="work", bufs=2) as wp, \
             tc.tile_pool(name="ppool", bufs=2, space="PSUM") as pp:

            # ---------------- loads ----------------
            SL = cp.tile([128, 8, NT], F32)
            sync.dma_start(out=SL[:], in_=locst[:])
            DY = cp.tile([128, 5, NT], F32)
            sync.dma_start(out=DY[:], in_=dyn[:])
            BBh = boxesT[:]
            BB = cp.tile([128, 4, NBOX], F32)
            bt_bc = bass.AP(tensor=BBh.tensor, offset=BBh.offset,
                            ap=[[0, 128], [NBOX, 4], [1, NBOX]])
            sync.dma_start(out=BB[:], in_=bt_bc)
            BP = cp.tile([2 * NBOX, 4], F32)
            sync.dma_start(out=BP[:], in_=boxesP[:])
            LV = cp.tile([128, 8, 5], F32)
            sync.dma_start(out=LV[:], in_=lvlc[:])
            # ---------------- per-box precompute ([128, 64] broadcast) -----
            from concourse.masks import make_identity
            IDT = cp.tile([128, 128], mybir.dt.bfloat16)
            make_identity(nc, IDT[:])
            x0, y0 = BB[:, 0, :], BB[:, 1, :]
            x1, y1 = BB[:, 2, :], BB[:, 3, :]
            CXY = cp.tile([128, 2, NBOX], F32)
            CX, CY = CXY[:, 0, :], CXY[:, 1, :]
            vec.tensor_tensor(out=CXY[:], in0=BB[:, 0:2, :], in1=BB[:, 2:4, :],
                              op=OP.add)
            vec.tensor_scalar(out=CXY[:], in0=CXY[:], scalar1=0.5, scalar2=None,
                              op0=OP.mult)
            # W2C = (w/2, h/2) per box; W2SQ = squared (bf16)
            W2C = cp.tile([128, 2, NBOX], F32)
            vec.tensor_tensor(out=W2C[:], in0=BB[:, 2:4, :], in1=BB[:, 0:2, :],
                              op=OP.subtract)
            vec.tensor_scalar(out=W2C[:], in0=W2C[:], scalar1=0.5, scalar2=None,
                              op0=OP.mult)
            W2SQ = cp.tile([128, 2, NBOX], mybir.dt.bfloat16)
            act.square(out=W2SQ[:], in_=W2C[:])
            S1 = cp.tile([128, NBOX], F32)
            S2 = cp.tile([128, NBOX], F32)
            # radius2 = max(K_R2 * area, 16);  IR2N = -1/radius2
            R2 = cp.tile([128, NBOX], F32)
            vec.tensor_tensor(out=S1[:], in0=x1, in1=x0, op=OP.subtract)  # w
            vec.tensor_tensor(out=S2[:], in0=y1, in1=y0, op=OP.subtract)  # h
            vec.tensor_tensor(out=R2[:], in0=S1[:], in1=S2[:], op=OP.mult)
            vec.tensor_scalar(out=R2[:], in0=R2[:], scalar1=K_R2,
                              scalar2=MIN_RADIUS2, op0=OP.mult, op1=OP.max)
            IR2N = cp.tile([128, NBOX], F32)
            vec.reciprocal(out=IR2N[:], in_=R2[:])
            vec.tensor_scalar(out=IR2N[:], in0=IR2N[:], scalar1=-1.0,
                              scalar2=None, op0=OP.mult)
            IR2NB = cp.tile([128, NBOX], mybir.dt.bfloat16)
            vec.tensor_copy(out=IR2NB[:], in_=IR2N[:])
            # S1 = w^2 + h^2 = (2*crit)^2 (cared tests done in squared domain)
            vec.tensor_tensor(out=S1[:], in0=S1[:], in1=S1[:], op=OP.mult)
            vec.tensor_tensor(out=S2[:], in0=S2[:], in1=S2[:], op=OP.mult)
            vec.tensor_tensor(out=S1[:], in0=S1[:], in1=S2[:], op=OP.add)
            # PBS [128, 4] = (x0, y0, -x1, -y1) boxes-on-partitions
            PBS = cp.tile([2 * NBOX, 4], F32)
            vec.tensor_copy(out=PBS[:, 0:2], in_=BP[:, 0:2])
            vec.tensor_scalar(out=PBS[:, 2:4], in0=BP[:, 2:4], scalar1=-1.0,
                              scalar2=None, op0=OP.mult)
            # bf16 hi/lo split of PBS (hi+lo covers f32 to ~2^-17 rel).
            # 5th column: (1, 0) so the one-hot matmul also yields the
            # match count (ties are averaged via the count column).
            BF16 = mybir.dt.bfloat16
            NSPL = 2
            PBS3 = cp.tile([NBOX, NSPL, 5], BF16)
            PR1 = cp.tile([NBOX, 4], F32)
            vec.memset(PBS3[:], 0.0)
            vec.memset(PBS3[:, 0, 4:5], 1.0)
            vec.tensor_copy(out=PBS3[:, 0, 0:4], in_=PBS[0:NBOX, :])
            vec.tensor_copy(out=PR1[:], in_=PBS3[:, 0, 0:4])  # hi back to f32
            vec.tensor_tensor(out=PR1[:], in0=PBS[0:NBOX, :], in1=PR1[:],
                              op=OP.subtract)
            vec.tensor_copy(out=PBS3[:, 1, 0:4], in_=PR1[:])
            # block-diagonal variant for paired-tile extraction:
            # rows 0:64 -> cols 0:5, rows 64:128 -> cols 5:10
            PBSD = cp.tile([128, NSPL, 10], BF16)
            vec.memset(PBSD[:], 0.0)
            vec.memset(PBSD[0:NBOX, 0, 4:5], 1.0)
            vec.memset(PBSD[NBOX:2 * NBOX, 0, 9:10], 1.0)
            SPL = cp.tile([128, NSPL, 4], BF16)
            PRF = cp.tile([128, 4], F32)
            vec.tensor_copy(out=SPL[:, 0, :], in_=PBS[:])
            vec.tensor_copy(out=PRF[:], in_=SPL[:, 0, :])
            vec.tensor_tensor(out=PRF[:], in0=PBS[:], in1=PRF[:],
                              op=OP.subtract)
            vec.tensor_copy(out=SPL[:, 1, :], in_=PRF[:])
            vec.tensor_copy(out=PBSD[0:NBOX, :, 0:4], in_=SPL[0:NBOX, :, :])
            vec.tensor_copy(out=PBSD[NBOX:2 * NBOX, :, 5:9],
                            in_=SPL[NBOX:2 * NBOX, :, :])
            # bf16 grid copy (lattice-exact: multiples of 4 <= 1020)
            BF16 = mybir.dt.bfloat16
            SLB = cp.tile([128, 2, NT], BF16)
            vec.tensor_copy(out=SLB[:], in_=SL[:, 0:2, :])
            # batched over levels: BCQ[l] = cared ? -1 : >=1 (bf16, max-combine);
            # CDALL = (cdisx, cdisy) per level, bf16 lattice-exact
            BCQ = []
            SF1 = cp.tile([128, NBOX], F32, tag="sf1", name="sf1")
            for l in range(5):
                lo, hi = SIZES[l]
                lo2, hi2 = 4.0 * lo * lo, 4.0 * hi * hi
                cn = cp.tile([128, NBOX], BF16, tag=f"bcq{l}", name=f"bcq{l}")
                vec.tensor_scalar(out=SF1[:], in0=S1[:], scalar1=lo2,
                                  scalar2=None, op0=OP.is_lt)
                vec.scalar_tensor_tensor(out=SF1[:], in0=S1[:],
                                         scalar=hi2, in1=SF1[:],
                                         op0=OP.is_gt, op1=OP.add)
                vec.tensor_scalar(out=cn[:], in0=SF1[:], scalar1=2.0,
                                  scalar2=-1.0, op0=OP.mult, op1=OP.add)
                BCQ.append(cn)

            def lvb(pl, shape):  # LV plane [128,5] -> bcast (128, d1, 5, 64)
                return (LV[:, pl, :].unsqueeze(1).broadcast_to((128, shape[1], 5))
                        .unsqueeze(3).broadcast_to(shape))

            B254 = (128, 2, 5, 64)
            UU = cp.tile([128, 2, 5, 64], F32, tag="uu", name="uu")
            R0 = cp.tile([128, 2, 5, 64], F32, tag="r0", name="r0")
            CC = cp.tile([128, 2, 5, 64], F32, tag="ccf", name="ccf")
            vec.tensor_tensor(
                out=UU[:], in0=CXY[:].unsqueeze(2).broadcast_to(B254),
                in1=lvb(0, B254), op=OP.mult)  # u = c/s
            vec.tensor_scalar(out=R0[:], in0=UU[:], scalar1=MAGIC,
                              scalar2=MAGIC, op0=OP.add, op1=OP.subtract)
            vec.tensor_tensor(out=CC[:], in0=R0[:], in1=UU[:], op=OP.is_gt)
            vec.tensor_tensor(out=R0[:], in0=R0[:], in1=CC[:], op=OP.subtract)
            CDALL = cp.tile([128, 2, 5, 64], BF16, tag="cdall", name="cdall")
            vec.tensor_tensor(out=R0[:], in0=R0[:], in1=lvb(1, B254),
                              op=OP.mult)
            vec.tensor_tensor(out=CDALL[:], in0=R0[:], in1=lvb(2, B254),
                              op=OP.add)  # floor(c/s)*s + s/2

            # ---------------- main pair loop --------------------------------
            # negated-min convention: plane 0 = -min(wdist2), 1 = -min(d)
            MINWD = cp.tile([128, 2, NT], BF16)
            # selected (x0, y0, -x1, -y1, count) sums over argmin one-hot
            XT5 = cp.tile([128, 5, NT], F32)

            for (t0, G, l) in SG:
                s = float(STRIDES[l])
                s2 = float(s * s)
                sl = slice(t0, t0 + G)

                def bb1(t2d):  # [128,64] const -> [128,G,64]
                    return t2d.unsqueeze(1).broadcast_to((128, G, 64))

                # DFC = (gx-cx, gy-cy) fp32
                DFC = wp.tile([128, 2, G, 64], F32, tag="dfc", name="dfc")
                vec.tensor_tensor(
                    out=DFC[:],
                    in0=SL[:, 0:2, sl].unsqueeze(3).broadcast_to((128, 2, G, 64)),
                    in1=CXY[:].unsqueeze(2).broadcast_to((128, 2, G, 64)),
                    op=OP.subtract)
                # DFD = (gx-cdisx, gy-cdisy) bf16, lattice-exact
                DFD = wp.tile([128, 2, G, 64], BF16, tag="dfd", name="dfd")
                vec.tensor_tensor(
                    out=DFD[:],
                    in0=SLB[:, :, sl].unsqueeze(3).broadcast_to((128, 2, G, 64)),
                    in1=CDALL[:, :, l, :].unsqueeze(2).broadcast_to(
                        (128, 2, G, 64)),
                    op=OP.subtract)
                SQ2 = wp.tile([128, 2, G, 64], BF16, tag="sq2", name="sq2")
                act.square(out=SQ2[:], in_=DFC[:])
                SQD = wp.tile([128, 2, G, 64], BF16, tag="sqd", name="sqd")
                act.square(out=SQD[:], in_=DFD[:])
                # in-box test in squared domain: dfx^2 >= (w/2)^2 -> outside
                MXQ = wp.tile([128, 2, G, 64], BF16, tag="mxq", name="mxq")
                vec.tensor_tensor(
                    out=MXQ[:], in0=SQ2[:],
                    in1=W2SQ[:].unsqueeze(2).broadcast_to((128, 2, G, 64)),
                    op=OP.subtract)
                M4 = wp.tile([128, G, 64], BF16, tag="m4", name="m4")
                vec.tensor_tensor(out=M4[:], in0=MXQ[:, 0], in1=MXQ[:, 1],
                                  op=OP.max)
                # peak / 3x3 tests on squared bf16 lattice values
                MQ = wp.tile([128, G, 64], BF16, tag="mq", name="mq")
                vec.tensor_tensor(out=MQ[:], in0=SQD[:, 0], in1=SQD[:, 1],
                                  op=OP.max)
                # invalid <=> max(MQ - 1.5*s^2, M4, BCQ) >= 0 (3x3 inclusive:
                # MQ lattice jumps s^2 -> 4s^2, so -1.5s^2 keeps MQ==s^2 valid)
                # V/PEN computed in place on M4's tile to save SBUF
                vec.scalar_tensor_tensor(out=M4[:], in0=MQ[:],
                                         scalar=-1.5 * s2, in1=M4[:],
                                         op0=OP.add, op1=OP.max)
                vec.tensor_tensor(out=M4[:], in0=M4[:], in1=bb1(BCQ[l][:]),
                                  op=OP.max)
                vec.tensor_scalar(out=M4[:], in0=M4[:], scalar1=0.0,
                                  scalar2=-INF, op0=OP.is_ge, op1=OP.mult)
                D2 = wp.tile([128, G, 64], BF16, tag="d2", name="d2")
                vec.tensor_tensor(out=D2[:], in0=SQ2[:, 0], in1=SQ2[:, 1],
                                  op=OP.add)
                vec.scalar_tensor_tensor(out=D2[:], in0=MQ[:], scalar=0.0,
                                         in1=D2[:], op0=OP.not_equal,
                                         op1=OP.mult)
                # WDN/DN share one tile -> single paired reduce
                WDD = wp.tile([128, 2, G, 64], BF16, tag="wdd", name="wdd")
                vec.tensor_tensor(out=WDD[:, 0], in0=D2[:], in1=bb1(IR2NB[:]),
                                  op=OP.mult)  # -wdist2
                vec.tensor_tensor(out=WDD[:, 1], in0=WDD[:, 0], in1=M4[:],
                                  op=OP.add)
                vec.tensor_reduce(out=MINWD[:, :, sl], in_=WDD[:], axis=AX.X,
                                  op=OP.max)
                # direct one-hot (ties summed; normalized later via count col)
                OH = wp.tile([128, G, 64], mybir.dt.bfloat16, tag="oh", name="oh")
                vec.tensor_tensor(out=OH[:], in0=WDD[:, 1],
                                  in1=MINWD[:, 1, sl].unsqueeze(2).broadcast_to(
                                      (128, G, 64)), op=OP.is_equal)
                # rt extraction on PE: paired-tile transpose + block-diag rhs
                if G % 2 == 0:
                    P2 = G // 2
                    OHT = pp.tile([128, P2, 128], mybir.dt.bfloat16, tag="oht",
                                  name="oht")
                    for gg in range(P2):
                        nc.tensor.transpose(
                            OHT[:, gg, :],
                            OH[:, 2 * gg:2 * gg + 2, :].rearrange("p a b -> p (a b)"), IDT[:])
                    OHTS = wp.tile([128, P2, 128], mybir.dt.bfloat16,
                                   tag="ohts", name="ohts")
                    act.copy(out=OHTS[:], in_=OHT[:])
                    RTP = pp.tile([128, G, 5], F32, tag="rtp", name="rtp")
                    for gg in range(P2):
                        for k in range(NSPL):
                            nc.tensor.matmul(
                                out=RTP[:, 2 * gg:2 * gg + 2, :].rearrange("p a b -> p (a b)"),
                                lhsT=OHTS[:, gg, :], rhs=PBSD[:, k, :],
                                start=(k == 0), stop=(k == NSPL - 1))
                else:
                    OHT = pp.tile([64, G, 128], mybir.dt.bfloat16, tag="oht1",
                                  name="oht1", bufs=1)
                    for g in range(G):
                        nc.tensor.transpose(OHT[:, g, :], OH[:, g, :], IDT[:])
                    OHTS = wp.tile([64, G, 128], mybir.dt.bfloat16,
                                   tag="ohts1", name="ohts1")
                    act.copy(out=OHTS[:], in_=OHT[:])
                    RTP = pp.tile([128, G, 5], F32, tag="rtp", name="rtp")
                    for g in range(G):
                        for k in range(NSPL):
                            nc.tensor.matmul(out=RTP[:, g, :],
                                             lhsT=OHTS[:, g, :],
                                             rhs=PBS3[:, k, :],
                                             start=(k == 0),
                                             stop=(k == NSPL - 1))
                act.copy(out=XT5[:, :, sl], in_=RTP[:].transpose([0, 2, 1]))

            # ---------------- epilogue: per-location [128, NT] --------------
            AGN = DY[:, 0, :]
            VAL = SL[:, 6, :]
            ISV = SL[:, 7, :]

            def lt(tag):
                return wp.tile([128, NT], F32, tag=tag, name=tag)

            HM = lt("hm")
            act.activation(out=HM[:], in_=MINWD[:, 0, :], func=AF.Exp, scale=1.0)
            vec.scalar_tensor_tensor(out=HM[:], in0=HM[:], scalar=SIG_LO,
                                     in1=HM[:], op0=OP.is_ge, op1=OP.mult)
            NW = lt("nw")
            vec.tensor_scalar(out=NW[:], in0=HM[:], scalar1=-1.0, scalar2=1.0,
                              op0=OP.mult, op1=OP.add)
            vec.tensor_tensor(out=NW[:], in0=NW[:], in1=NW[:], op=OP.mult)
            vec.tensor_tensor(out=NW[:], in0=NW[:], in1=NW[:], op=OP.mult)
            PC = lt("pc")
            act.activation(out=PC[:], in_=AGN, func=AF.Sigmoid)
            vec.tensor_scalar(out=PC[:], in0=PC[:], scalar1=SIG_LO,
                              scalar2=SIG_HI, op0=OP.max, op1=OP.min)
            Q = lt("q")
            vec.tensor_scalar(out=Q[:], in0=PC[:], scalar1=-1.0, scalar2=1.0,
                              op0=OP.mult, op1=OP.add)
            act.activation(out=Q[:], in_=Q[:], func=AF.Ln)  # log(1-pred)
            P2 = lt("p2")
            vec.tensor_tensor(out=P2[:], in0=PC[:], in1=PC[:], op=OP.mult)
            T1 = lt("t1")
            vec.tensor_tensor(out=T1[:], in0=Q[:], in1=P2[:], op=OP.mult)
            vec.tensor_tensor(out=T1[:], in0=T1[:], in1=NW[:], op=OP.mult)
            GT = lt("gt")
            vec.tensor_scalar(out=GT[:], in0=PC[:], scalar1=IGNORE_HIGH_FP,
                              scalar2=None, op0=OP.is_lt)
            vec.tensor_tensor(out=T1[:], in0=T1[:], in1=GT[:], op=OP.mult)
            vec.tensor_tensor(out=T1[:], in0=T1[:], in1=VAL, op=OP.mult)
            NEGA = cp.tile([128, 1], F32)
            vec.tensor_reduce(out=NEGA[:], in_=T1[:], axis=AX.X, op=OP.add)
            # validity + rt
            VM = lt("vm")
            vec.tensor_scalar(out=VM[:], in0=MINWD[:, 1, :], scalar1=-INF / 2,
                              scalar2=None, op0=OP.is_gt)
            vec.tensor_tensor(out=VM[:], in0=VM[:], in1=VAL, op=OP.mult)
            REGC = cp.tile([128, 1], F32)
            vec.tensor_reduce(out=REGC[:], in_=VM[:], axis=AX.X, op=OP.add)
            # normalize one-hot sums by match count (ties averaged)
            RCPC = lt("rcpc")
            vec.reciprocal(out=RCPC[:], in_=XT5[:, 4, :])
            XT = wp.tile([128, 4, NT], F32, tag="xt", name="xt")
            vec.tensor_tensor(out=XT[:], in0=XT5[:, 0:4, :],
                              in1=RCPC[:].unsqueeze(1).broadcast_to((128, 4, NT)),
                              op=OP.mult)
            RT = wp.tile([128, 4, NT], F32, tag="rt", name="rt")
            vec.scalar_tensor_tensor(out=RT[:, 0:2, :], in0=XT[:, 0:2, :],
                                     scalar=-1.0, in1=SL[:, 0:2, :],
                                     op0=OP.mult, op1=OP.add)
            vec.scalar_tensor_tensor(out=RT[:, 2:4, :], in0=XT[:, 2:4, :],
                                     scalar=-1.0, in1=SL[:, 4:6, :],
                                     op0=OP.mult, op1=OP.add)
            # RT = signed_grid - XT = (l, t, r, b) of argmin box
            vec.tensor_tensor(out=RT[:], in0=RT[:],
                              in1=ISV.unsqueeze(1).broadcast_to((128, 4, NT)),
                              op=OP.mult)
            # rtf = rt*vm + (1-vm)   (exact select; vm in {0,1})
            RTF = wp.tile([128, 4, NT], F32, tag="rtf", name="rtf")
            vec.tensor_tensor(out=RTF[:], in0=RT[:],
                              in1=VM[:].unsqueeze(1).broadcast_to((128, 4, NT)),
                              op=OP.mult)
            VMN = lt("vmn")
            vec.tensor_scalar(out=VMN[:], in0=VM[:], scalar1=-1.0, scalar2=1.0,
                              op0=OP.mult, op1=OP.add)
            vec.tensor_tensor(out=RTF[:], in0=RTF[:],
                              in1=VMN[:].unsqueeze(1).broadcast_to((128, 4, NT)),
                              op=OP.add)
            # giou(pred, rtf)
            pl, pt = DY[:, 1, :], DY[:, 2, :]
            pr, pb = DY[:, 3, :], DY[:, 4, :]
            tl, tt_ = RTF[:, 0, :], RTF[:, 1, :]
            tr, tb = RTF[:, 2, :], RTF[:, 3, :]
            TA, PA, WI, GW, HI, GH = (lt("ta"), lt("pa"), lt("wi"), lt("gw"),
                                      lt("hi"), lt("gh"))
            SA, SB = lt("sa"), lt("sb")
            vec.tensor_tensor(out=SA[:], in0=tl, in1=tr, op=OP.add)
            vec.tensor_tensor(out=SB[:], in0=tt_, in1=tb, op=OP.add)
            vec.tensor_tensor(out=TA[:], in0=SA[:], in1=SB[:], op=OP.mult)
            vec.tensor_tensor(out=SA[:], in0=pl, in1=pr, op=OP.add)
            vec.tensor_tensor(out=SB[:], in0=pt, in1=pb, op=OP.add)
            vec.tensor_tensor(out=PA[:], in0=SA[:], in1=SB[:], op=OP.mult)
            vec.tensor_tensor(out=SA[:], in0=pl, in1=tl, op=OP.min)
            vec.tensor_tensor(out=SB[:], in0=pr, in1=tr, op=OP.min)
            vec.tensor_tensor(out=WI[:], in0=SA[:], in1=SB[:], op=OP.add)
            vec.tensor_tensor(out=SA[:], in0=pl, in1=tl, op=OP.max)
            vec.tensor_tensor(out=SB[:], in0=pr, in1=tr, op=OP.max)
            vec.tensor_tensor(out=GW[:], in0=SA[:], in1=SB[:], op=OP.add)
            vec.tensor_tensor(out=SA[:], in0=pb, in1=tb, op=OP.min)
            vec.tensor_tensor(out=SB[:], in0=pt, in1=tt_, op=OP.min)
            vec.tensor_tensor(out=HI[:], in0=SA[:], in1=SB[:], op=OP.add)
            vec.tensor_tensor(out=SA[:], in0=pb, in1=tb, op=OP.max)
            vec.tensor_tensor(out=SB[:], in0=pt, in1=tt_, op=OP.max)
            vec.tensor_tensor(out=GH[:], in0=SA[:], in1=SB[:], op=OP.add)
            AC = lt("ac")
            vec.tensor_tensor(out=AC[:], in0=GW[:], in1=GH[:], op=OP.mult)
            vec.tensor_scalar(out=AC[:], in0=AC[:], scalar1=EPS_AC,
                              scalar2=None, op0=OP.add)
            INTER = lt("inter")
            vec.tensor_tensor(out=INTER[:], in0=WI[:], in1=HI[:], op=OP.mult)
            UN = lt("un")
            vec.tensor_tensor(out=UN[:], in0=TA[:], in1=PA[:], op=OP.add)
            vec.tensor_tensor(out=UN[:], in0=UN[:], in1=INTER[:], op=OP.subtract)
            vec.tensor_scalar(out=SA[:], in0=INTER[:], scalar1=1.0,
                              scalar2=None, op0=OP.add)
            vec.tensor_scalar(out=SB[:], in0=UN[:], scalar1=1.0,
                              scalar2=None, op0=OP.add)
            IOU = lt("iou")
            vec.reciprocal(out=SB[:], in_=SB[:])
            vec.tensor_tensor(out=IOU[:], in0=SA[:], in1=SB[:], op=OP.mult)
            vec.tensor_tensor(out=SA[:], in0=AC[:], in1=UN[:], op=OP.subtract)
            vec.reciprocal(out=SB[:], in_=AC[:])
            vec.tensor_tensor(out=SB[:], in0=SA[:], in1=SB[:], op=OP.mult)
            vec.tensor_tensor(out=IOU[:], in0=IOU[:], in1=SB[:], op=OP.subtract)
            vec.tensor_scalar(out=IOU[:], in0=IOU[:], scalar1=-1.0, scalar2=1.0,
                              op0=OP.mult, op1=OP.add)  # 1 - giou
            vec.tensor_tensor(out=IOU[:], in0=IOU[:], in1=VM[:], op=OP.mult)
            REGA = cp.tile([128, 1], F32)
            vec.tensor_reduce(out=REGA[:], in_=IOU[:], axis=AX.X, op=OP.add)

            # ---------------- partial reduction (host finishes) -------------
            PART = cp.tile([128, 8], F32)
            vec.memset(PART[:], 0.0)
            vec.tensor_copy(out=PART[:, 0:1], in_=REGA[:])
            vec.tensor_copy(out=PART[:, 1:2], in_=REGC[:])
            vec.tensor_copy(out=PART[:, 3:4], in_=NEGA[:])
            ONES = cp.tile([128, 1], F32)
            vec.memset(ONES[:], 1.0)
            PS = pp.tile([1, 8], F32, bufs=1)
            nc.tensor.matmul(out=PS[:], lhsT=ONES[:], rhs=PART[:],
                             start=True, stop=True)
            PSB = cp.tile([1, 8], F32)
            vec.tensor_copy(out=PSB[:], in_=PS[:])
            if dbg:
                sync.dma_start(out=minddbg[:], in_=MINWD[:, 1, :])
                sync.dma_start(out=minwdbg[:], in_=MINWD[:, 0, :])
                sync.dma_start(out=xtdbg[:], in_=XT5[:, 0:4, :])
            sync.dma_start(out=out[:], in_=PSB[:])
    nc.compile()
    return nc


# ------------------------------ host wrapper -------------------------------

def make_in_maps(boxes, agn_hm_pred, reg_pred):
    boxes = np.ascontiguousarray(np.asarray(boxes, np.float32))
    agn = np.ascontiguousarray(np.asarray(agn_hm_pred, np.float32))
    rp = np.ascontiguousarray(np.asarray(reg_pred, np.float32))
    in_maps = []
    for c in range(N_CORES):
        b, h = c // 2, c % 2
        idx = _SHARD_IDX[(b, h)]
        dyn = np.zeros((128, 5, NT), np.float32)
        a = np.zeros(NPAD, np.float32)
        a[:NV] = agn[idx]
        dyn[:, 0, :] = _pack(a)
        r = np.zeros((NPAD, 4), np.float32)
        r[:NV] = rp[idx]
        for k in range(4):
            dyn[:, 1 + k, :] = _pack(np.ascontiguousarray(r[:, k]))
        in_maps.append({
            "locst": _LOCSTAT[h],
            "dyn": np.ascontiguousarray(dyn),
            "boxesT": np.ascontiguousarray(boxes[b].T),
            "boxesP": np.ascontiguousarray(np.tile(boxes[b], (2, 1))),
            "lvlc": _LVLC,
        })
    return in_maps


_NC_CACHE = {}
LAST_RESULT = None


def _get_nc():
    if "nc" not in _NC_CACHE:
        _NC_CACHE["nc"] = build_nc(dbg=False)
    return _NC_CACHE["nc"]


def kernel(boxes, gt_classes=None, agn_hm_pred=None, reg_pred=None):
    global LAST_RESULT
    in_maps = make_in_maps(boxes, agn_hm_pred, reg_pred)
    nc = _get_nc()
    res = run_bass_kernel_spmd(nc, in_maps, core_ids=list(range(N_CORES)))
    LAST_RESULT = res
    red = np.zeros(8, np.float64)
    for r in res.results:
        red += np.asarray(r["out"], np.float32).reshape(8).astype(np.float64)
    rega, regc, nega = red[0], red[1], red[3]
    # positive-sample focal term on host (tiny: [B, N, 5] indexing prep)
    bx = np.asarray(boxes, np.float32)
    agn = np.asarray(agn_hm_pred, np.float32).reshape(-1)
    centers = (bx[..., :2] + bx[..., 2:]) / 2
    st = np.asarray(STRIDES, np.float32)
    ci = (centers[:, :, None, :] / st[None, None, :, None]).astype(np.int64)
    Ws = np.asarray([w for (_, w) in LEVEL_HW], np.int64)
    locv = np.asarray(LOC, np.int64)
    basev = np.asarray(BASE, np.int64)
    im = np.arange(B)[:, None, None]
    pos = (basev[None, None, :] + im * locv[None, None, :]
           + ci[..., 1] * Ws[None, None, :] + ci[..., 0])
    idx = np.clip(pos.reshape(-1), 0, M_TOT - 1)
    crit = np.sqrt(((bx[..., 2:] - bx[..., :2]) ** 2).sum(-1)) / 2
    lov = np.asarray([r[0] for r in SIZES])
    hiv = np.asarray([r[1] for r in SIZES])
    mask = ((crit[..., None] >= lov) & (crit[..., None] <= hiv)).reshape(-1)
    pred = np.clip(1.0 / (1.0 + np.exp(-agn[idx].astype(np.float64))),
                   1e-4, 1.0 - 1e-4)
    poss = float(np.where(mask, np.log(pred) * (1.0 - pred) ** 2, 0.0).sum())
    npa = max(float(mask.sum()), 1.0)
    reg_loss = rega / max(regc, 1.0)
    agn_pos = -0.125 * poss / npa
    agn_neg = -0.375 * nega / npa
    return np.array([reg_loss, agn_pos, agn_neg], np.float32)

